# revision 43
# baseline (speedup 1.0000x reference)
"""AdaLN self-attention block (B=2, L=2048, C=1024, H=16, DFF=4096) on 8 TRN2 cores.

Sharding: DP=2 over batch (cores 0-3 -> batch 0, cores 4-7 -> batch 1),
sequence-parallel 4-way within each group (512 query tokens per core).
Each core holds full weights, computes q/k/v for its own 512 tokens,
all-gathers normalized K and V (with an appended ones column for the softmax
denominator) within its 4-core group, runs full attention for its queries,
then proj + FFN locally on its token slice. Host concatenates the slices.

Dense matmuls (qkv / v / proj / fc1 / fc2) run in fp8e4m3 with DoubleRow
perf mode (256-deep contraction per pass, ~1.7x PE throughput). Weights are
scaled x64 on the host so they sit in e4m3's normal range; the descale by
1/64 is folded into each consumer's existing scalar op. Activations feeding
those matmuls (h1, h2, attn, gelu) are written as fp8 directly by their
producing ops. Attention QK / AV and the residual stream stay bf16.

Everything on-chip is feature-major ([C, tokens]); the host pre-transposes
activations/weights so no on-device transposes are needed (except a tiny
48x128 one for the adaLN modulation vector).
"""

import os
import sys

for _p in ("/opt/trn_rl_repo", os.path.expanduser("~/.axon_site/_ro/trn_rl_repo")):
    if os.path.isdir(_p) and _p not in sys.path:
        sys.path.insert(0, _p)

import numpy as np
import ml_dtypes

import concourse.bass as bass
import concourse.tile as tile
from concourse import mybir
from concourse.bass import ds, ts
from concourse import bass_utils

BF16 = mybir.dt.bfloat16
F32 = mybir.dt.float32
F8 = mybir.dt.float8e4
AF = mybir.ActivationFunctionType
DR = mybir.MatmulPerfMode.DoubleRow

B, L, C, H, DH, DFF, D = 2, 2048, 1024, 16, 64, 4096, 1024
NCORES = 8
GROUP = 4          # cores per batch group
T = L // GROUP     # 512 query tokens per core
CT = C // 128      # 8 feature tiles
KK = C // 256      # 4 DoubleRow contraction slabs over C
ADA_SLICE = 6 * C // GROUP  # 1536 adaLN outputs per core
EPS = 1e-6
MAX_SCALE_MUL = float(np.log(100.0))
WS = 64.0          # fp8 weight scale
REPLICA_GROUPS = [[0, 1, 2, 3], [4, 5, 6, 7]]

_CACHE = {}


# --------------------------------------------------------------------------- #
# graph construction
# --------------------------------------------------------------------------- #

def _build(sim_gelu=False, split_waits=True):
    nc = bass.Bass(
        "TRN2", target_bir_lowering=False, debug=False, num_devices=NCORES
    )

    def inp(name, shape, dt):
        return nc.dram_tensor(name, shape, dt, kind="ExternalInput").ap()

    p = {
        "xb": inp("xb", [CT, 128, T], BF16),      # x^T slice, tiled, bf16
        "cond8": inp("cond8", [128, 8], F32),     # cond feature-major
        "biasT": inp("biasT", [16, 128, T], BF16),  # exp-bias source, tiled
        "qkw8": inp("qkw8", [KK, 128, 2, 2 * C], F8),
        "vw8": inp("vw8", [KK, 128, 2, C], F8),
        "projw8": inp("projw8", [KK, 128, 2, C], F8),
        "fc1w8": inp("fc1w8", [KK, 128, 2, DFF], F8),
        "fc2w8": inp("fc2w8", [DFF // 256, 128, 2, C], F8),
        "adawT": inp("adawT", [6, CT, 128, 1024], BF16),
        "adab48": inp("adab48", [128, 48], F32),
        "qb8": inp("qb8", [128, CT], F32),
        "vb2": inp("vb2", [1, C], BF16),          # host-scaled x WS
        "pb8": inp("pb8", [128, CT], F32),
        "f1b": inp("f1b", [128, DFF // 128], F32),
        "f2b": inp("f2b", [128, CT], F32),
        "smv": inp("smv", [16, 1], F32),
        "ones128": inp("ones128", [128, 128], BF16),
        "hsel": inp("hsel", [128, CT, 16], BF16),
        "hselT": inp("hselT", [16, CT, 128], BF16),
        "ones1_128": inp("ones1_128", [1, 128], BF16),
        "pairsel": inp("pairsel", [2, 128], BF16),
        "eye48": inp("eye48", [48, 48], F32),
    }
    out = nc.dram_tensor("out", [C, T], F32, kind="ExternalOutput").ap()

    with tile.TileContext(nc) as tc:
        _emit(nc, tc, p, out, sim_gelu)
    if split_waits:
        _split_waits(nc)
    return nc


_SPLIT_TYPES = {
    "InstTensorTensor", "InstTensorScalarPtr", "InstReciprocal",
    "InstTensorCopy", "InstActivation", "InstTensorReduce", "InstMemset",
    "InstMatmult", "InstLdweights", "InstCopyPredicated", "InstBnStats",
    "InstBnAggr", "InstStreamTranspose", "InstDMACopy", "InstDrain",
    "InstCollectiveCompute",
}


def _split_waits(nc, max_waits=1):
    """Walrus TPB codegen rejects >1 sync-wait on compute instructions;
    hoist extras onto standalone EventSemaphore waits on the same engine."""
    for bb in nc.main_func.blocks:
        new = []
        changed = False
        for ins in bb.instructions:
            si = getattr(ins, "sync_info", None)
            if (
                si is not None
                and si.on_wait
                and len(si.on_wait) > max_waits
                and type(ins).__name__ in _SPLIT_TYPES
            ):
                waits = list(si.on_wait)
                for i, w in enumerate(waits[:-max_waits]):
                    ws = mybir.InstEventSemaphore(
                        name=f"{ins.name}_w{i}", ins=[], outs=[]
                    )
                    ws.engine = ins.engine
                    ws.sync_info = mybir.SyncInfo(on_wait=[w], on_update=[])
                    new.append(ws)
                ins.sync_info = mybir.SyncInfo(
                    on_wait=waits[-max_waits:], on_update=list(si.on_update)
                )
                changed = True
            new.append(ins)
        if changed:
            bb.instructions = new


def _emit(nc, tc, p, out_d, sim_gelu=False):

    # ---- persistent SBUF pools -------------------------------------------- #
    const = tc.alloc_tile_pool(name="const", bufs=1)
    persist = tc.alloc_tile_pool(name="persist", bufs=1)
    work = tc.alloc_tile_pool(name="work", bufs=4)
    stats = tc.alloc_tile_pool(name="stats", bufs=1)
    wpool = tc.alloc_tile_pool(name="wpool", bufs=1)
    dram = tc.alloc_tile_pool(name="dram", bufs=1, space="DRAM")

    # ---- constants / small inputs to SBUF --------------------------------- #
    def load_const(name, shape, dt):
        t = const.tile(shape, dt, tag=name, name=name)
        nc.sync.dma_start(out=t[:], in_=p[name])
        return t

    # ada path inputs first: its collective is the first serialization point
    cond8 = load_const("cond8", [128, 8], F32)
    adab48 = load_const("adab48", [128, 48], F32)
    eye48 = load_const("eye48", [48, 48], F32)
    ones128 = load_const("ones128", [128, 128], BF16)
    hsel = load_const("hsel", [128, CT, 16], BF16)
    hselT = load_const("hselT", [16, CT, 128], BF16)
    ones1_128 = load_const("ones1_128", [1, 128], BF16)
    pairsel = load_const("pairsel", [2, 128], BF16)
    qb8 = load_const("qb8", [128, CT], F32)
    vb2 = load_const("vb2", [1, C], BF16)
    pb8 = load_const("pb8", [128, CT], F32)
    f1b = load_const("f1b", [128, DFF // 128], F32)
    f2b = load_const("f2b", [128, CT], F32)
    smv_in = load_const("smv", [16, 1], F32)

    # ---- DRAM bounce buffers ---------------------------------------------- #
    ada_l = dram.tile([1, 6 * C], F32, tag="ada_l")
    k_in = dram.tile([C, T], BF16, tag="k_in")
    k_gh = dram.tile([2, GROUP, 4, 128, T], BF16, tag="k_gh")
    v_in2 = dram.tile([H // 2, T, 2, DH + 1], BF16, tag="v_in2")
    v_gh = dram.tile([2, GROUP, 4, T, 2, DH + 1], BF16, tag="v_gh")
    rq_d = dram.tile([16, T], BF16, tag="rq_d")
    rk_d = dram.tile([16, T], BF16, tag="rk_d")
    den_d = dram.tile([16, T], F32, tag="den_d")

    # ============================ phase 1 PSUM ============================= #
    ps1 = tc.alloc_tile_pool(name="ps1", bufs=1, space="PSUM")

    # ---- adaLN: silu(cond) @ ada_w^T slice, then group all-gather --------- #
    sig = work.tile([128, 8], F32, tag="w8")
    nc.scalar.activation(out=sig[:], in_=cond8[:], func=AF.Exp, scale=-1.0)
    nc.vector.tensor_scalar_add(out=sig[:], in0=sig[:], scalar1=1.0)
    nc.vector.reciprocal(out=sig[:], in_=sig[:])
    silu = work.tile([128, 8], BF16, tag="w8b")
    nc.vector.tensor_tensor(
        out=silu[:], in0=sig[:], in1=cond8[:], op=mybir.AluOpType.mult
    )

    # Every core computes the full 6C adaLN vector redundantly: a collective
    # here costs ~55us of trigger latency, the redundant matmuls only ~25us.
    _sc_ada = nc.named_scope("ada"); _sc_ada.__enter__()
    for ng in range(6):
        aps = [ps1.tile([1, 512], F32, tag="sm", bufs=2, name=f"aps{ng}_{i}")
               for i in range(2)]
        for k in range(CT):
            wt = wpool.tile([128, 1024], BF16, tag="wada", bufs=2, name="wada")
            # scalar HWDGE ring: keeps 48 issues off the busy sync ring
            nc.scalar.dma_start(out=wt[:], in_=p["adawT"][ng, k])
            for i in range(2):
                nc.tensor.matmul(
                    aps[i][:], silu[:, ds(k, 1)], wt[:, ds(512 * i, 512)],
                    start=(k == 0), stop=(k == CT - 1),
                )
        for i in range(2):
            aw = work.tile([1, 512], F32, tag="w1x512", name="aw")
            nc.vector.tensor_copy(out=aw[:], in_=aps[i][:])
            nc.sync.dma_start(
                out=ada_l[0, ds(1024 * ng + 512 * i, 512)], in_=aw[:]
            )
    # load [48,128] token-major, transpose on PE -> mod [128, 48]
    mod = persist.tile([128, 48], F32, tag="mod")
    ada_tm = work.tile([48, 128], F32, tag="ada_tm")
    nc.sync.dma_start(out=ada_tm[:], in_=ada_l.rearrange("g n -> (g n)").rearrange("(j p) -> j p", p=128))
    modps = ps1.tile([128, 48], F32, tag="sm", bufs=2)
    nc.tensor.transpose(modps[:], ada_tm[:], eye48[:])
    nc.vector.tensor_tensor(out=mod[:], in0=modps[:], in1=adab48[:],
                            op=mybir.AluOpType.add)
    # s1, s2 chunks get +1
    nc.vector.tensor_scalar_add(out=mod[:, 16:32], in0=mod[:, 16:32], scalar1=1.0)
    # descaled copies of g1 / g2 columns for fp8 PSUM consumers
    modg = stats.tile([128, 16], F32, tag="modg")
    nc.vector.tensor_scalar_mul(out=modg[:], in0=mod[:, 0:16], scalar1=1.0 / WS)
    pbg1 = stats.tile([128, CT], F32, tag="pbg1")
    nc.vector.tensor_tensor(out=pbg1[:], in0=pb8[:], in1=mod[:, 0:8],
                            op=mybir.AluOpType.mult)
    fbg2 = stats.tile([128, CT], F32, tag="fbg2")
    nc.vector.tensor_tensor(out=fbg2[:], in0=f2b[:], in1=mod[:, 8:16],
                            op=mybir.AluOpType.mult)
    _sc_ada.__exit__(None, None, None)

    # ---- persistent activations ------------------------------------------- #
    xb = persist.tile([128, CT, T], BF16, tag="big_d")       # x^T bf16
    nc.sync.dma_start(out=xb[:], in_=p["xb"].rearrange("t p q -> p t q"))

    h1 = persist.tile([128, CT, T], F8, tag="big_a")         # LN1-mod, fp8
    qe = persist.tile([128, CT, T], BF16, tag="big_b")       # q (later normed)
    ke = persist.tile([128, CT, T], BF16, tag="big_c")       # k (later normed)
    v_pre = persist.tile([128, T // 128, H, DH + 1], BF16, tag="vpre")
    eb = persist.tile([128, L // 128, T], BF16, tag="eb")    # exp(bias^T)
    attn = persist.tile([128, CT, T], F8, tag="big_a")       # fp8 probs@V
    anum = persist.tile([128, CT, T], BF16, tag="vpre")      # unscaled attn out
    x2 = persist.tile([128, CT, T], BF16, tag="big_c")
    h2 = persist.tile([128, CT, T], F8, tag="big_b")
    gact = persist.tile([128, DFF // 128, T], F8, tag="big_d")
    kfull = persist.tile([128, CT, GROUP, T], BF16, tag="kfull")

    # ---- expbias (independent; emitted early so it overlaps) -------------- #
    nc.sync.dma_start(out=eb[:], in_=p["biasT"].rearrange("t p q -> p t q"))
    for i in range(4):
        nc.scalar.activation(
            out=eb[:, ds(4 * i, 4), :], in_=eb[:, ds(4 * i, 4), :], func=AF.Exp
        )

    # ---- scale_mul -> smv = exp(min(scale_mul, log 100)) ------------------ #
    eps128 = const.tile([128, 1], F32, tag="eps128")
    nc.vector.memset(eps128[:], EPS)
    smv = stats.tile([16, 1], F32, tag="smv")
    nc.vector.tensor_scalar_min(out=smv[:], in0=smv_in[:], scalar1=MAX_SCALE_MUL)
    nc.scalar.activation(out=smv[:], in_=smv[:], func=AF.Exp)

    # ---- layernorm helper (feature-major, partition sums via ones matmul) - #
    def layernorm(src, dst, s_col, sh_col, psp, mm_bufs):
        s1 = psp.tile([128, T], F32, tag="mm", bufs=mm_bufs)
        s2 = psp.tile([128, T], F32, tag="mm", bufs=mm_bufs)
        for t in range(CT):
            sq = work.tile([128, T], BF16, tag="sq")
            nc.vector.tensor_tensor(
                out=sq[:], in0=src[:, t, :], in1=src[:, t, :], op=mybir.AluOpType.mult
            )
            nc.tensor.matmul(s1[:], ones128[:], src[:, t, :],
                             start=(t == 0), stop=(t == CT - 1), skip_group_check=True)
            nc.tensor.matmul(s2[:], ones128[:], sq[:],
                             start=(t == 0), stop=(t == CT - 1), skip_group_check=True)
        meanb = stats.tile([128, T], F32, tag="meanb")
        nc.vector.tensor_scalar_mul(out=meanb[:], in0=s1[:], scalar1=1.0 / C)
        m2 = stats.tile([128, T], F32, tag="m2")
        nc.vector.tensor_tensor(out=m2[:], in0=meanb[:], in1=meanb[:],
                                op=mybir.AluOpType.mult)
        varb = stats.tile([128, T], F32, tag="varb")
        nc.vector.scalar_tensor_tensor(
            out=varb[:], in0=s2[:], scalar=1.0 / C, in1=m2[:],
            op0=mybir.AluOpType.mult, op1=mybir.AluOpType.subtract,
        )
        # rstd = exp(-0.5 * ln(var + eps))   (stays in the exp/ln table set)
        nc.scalar.activation(out=varb[:], in_=varb[:], func=AF.Ln, bias=eps128[:])
        rstdb = stats.tile([128, T], F32, tag="rstdb")
        nc.scalar.activation(out=rstdb[:], in_=varb[:], func=AF.Exp, scale=-0.5)
        for t in range(CT):
            d1 = work.tile([128, T], F32, tag="d1")
            nc.vector.tensor_tensor(out=d1[:], in0=src[:, t, :], in1=meanb[:],
                                    op=mybir.AluOpType.subtract)
            nc.vector.tensor_tensor(out=d1[:], in0=d1[:], in1=rstdb[:],
                                    op=mybir.AluOpType.mult)
            nc.vector.tensor_scalar(
                out=dst[:, t, :], in0=d1[:],
                scalar1=mod[:, ds(s_col + t, 1)], scalar2=mod[:, ds(sh_col + t, 1)],
                op0=mybir.AluOpType.mult, op1=mybir.AluOpType.add,
            )

    with nc.named_scope("ln1"):
        layernorm(xb, h1, 16, 32, ps1, 4)  # s1 cols 16..23, sh1 cols 32..39
    # fold proj_b*g1 into the residual now so proj_consume is a single op
    for t in range(CT):
        nc.vector.tensor_scalar_add(out=xb[:, t, :], in0=xb[:, t, :],
                                    scalar1=pbg1[:, ds(t, 1)])

    # ---- qkv weights: 4 resident fp8 slabs -------------------------------- #
    qkw = []
    for kk in range(KK):
        # tag shared with the fc1 slabs (same shape/dtype, disjoint lifetime)
        wt = wpool.tile([128, 2, 2 * C], F8, tag=f"wbig{kk}", name="wqk")
        nc.sync.dma_start(out=wt[:], in_=p["qkw8"][kk])
        qkw.append(wt)

    # ---- qkv: K first (so its all-gather overlaps V and Q compute) ------- #
    _sc_qkv = nc.named_scope("qkv"); _sc_qkv.__enter__()
    ssq_q = ps1.tile([16, T], F32, tag="ss", bufs=2)
    ssq_k = ps1.tile([16, T], F32, tag="ss", bufs=2)

    def qk_block(ms, is_q):
        # ms: global m-tile indices into the 2C q/k output (0..7 q, 8..15 k)
        for mg in range(0, len(ms), 4):
            sub = ms[mg:mg + 4]
            accs = [ps1.tile([128, T], F32, tag="mm", bufs=4, name=f"qk{m}")
                    for m in sub]
            for kk in range(KK):
                for i, m in enumerate(sub):
                    nc.tensor.matmul(
                        accs[i][:], qkw[kk][:, :, ds(128 * m, 128)],
                        h1[:, ds(2 * kk, 2), :],
                        start=(kk == 0), stop=(kk == KK - 1), perf_mode=DR,
                    )
            for i, m in enumerate(sub):
                acc = accs[i]
                if is_q:
                    dst = qe[:, m, :]
                    nc.vector.tensor_scalar(
                        out=dst, in0=acc[:], scalar1=1.0 / WS,
                        scalar2=qb8[:, ds(m, 1)],
                        op0=mybir.AluOpType.mult, op1=mybir.AluOpType.add,
                    )
                else:
                    dst = ke[:, m - 8, :]
                    nc.vector.tensor_scalar_mul(out=dst, in0=acc[:],
                                                scalar1=1.0 / WS)
                sq = work.tile([128, T], BF16, tag="sq")
                nc.vector.tensor_tensor(out=sq[:], in0=dst, in1=dst,
                                        op=mybir.AluOpType.mult)
                tgt = ssq_q if is_q else ssq_k
                tm = m % 8
                nc.tensor.matmul(tgt[:], hsel[:, tm, :], sq[:],
                                 start=(tm == 0), stop=(tm == 7),
                                 skip_group_check=True)

    def make_rnorm(ssq, with_sm):
        r = stats.tile([16, T], F32, tag="rn_f")
        nc.vector.tensor_scalar_max(out=r[:], in0=ssq[:], scalar1=1e-24)
        nc.scalar.activation(out=r[:], in_=r[:], func=AF.Ln)
        rb = stats.tile([16, T], BF16, tag="rn_bq" if with_sm else "rn_bk", name="rb")
        nc.scalar.activation(out=rb[:], in_=r[:], func=AF.Exp, scale=-0.5)
        if with_sm:
            nc.vector.tensor_scalar_mul(out=rb[:], in0=rb[:], scalar1=smv[:])
        return rb

    def rnorm_apply(rb, rd_bounce, dst, psp):
        # partition remap [16,T] -> [2,8,T] via a DRAM roundtrip, then a
        # K=2 pairsel matmul broadcasts each head row over its 64 partitions
        nc.sync.dma_start(out=rd_bounce[:], in_=rb[:])
        rn2 = work.tile([2, 8, T], BF16, tag="rn2", bufs=1, name="rn2")
        nc.sync.dma_start(out=rn2[:],
                          in_=rd_bounce.rearrange("(t j) q -> j t q", j=2))
        for t in range(CT):
            bc = psp.tile([128, T], F32, tag="sm", bufs=2, name="bcn")
            nc.tensor.matmul(bc[:], pairsel[:], rn2[:, t, :], start=True, stop=True)
            nc.vector.tensor_tensor(out=dst[:, t, :], in0=dst[:, t, :], in1=bc[:],
                                    op=mybir.AluOpType.mult)

    qk_block(list(range(8, 16)), False)  # K tiles
    rkn = make_rnorm(ssq_k, False)
    rnorm_apply(rkn, rk_d, ke, ps1)
    _sc_qkv.__exit__(None, None, None)

    # ---- K all-gather, split in halves so attention can start early ------- #
    kin_t = k_in.rearrange("(t p) q -> t p q", p=128)
    with nc.named_scope("agK"):
        for h in range(2):
            for t in range(4 * h, 4 * h + 4):
                nc.sync.dma_start(out=kin_t[t], in_=ke[:, t, :])
            nc.gpsimd.collective_compute(
                "AllGather", mybir.AluOpType.bypass,
                replica_groups=REPLICA_GROUPS,
                ins=[k_in[ds(512 * h, 512), :].opt()], outs=[k_gh[h].opt()],
            )
    for t in range(CT):
        h, tl = divmod(t, 4)
        nc.sync.dma_start(
            out=kfull[:, t, :, :],
            in_=k_gh[h, :, tl].rearrange("r p q -> p r q"),
        )

    # ---- V (token-major) + ones column, then all-gather ------------------- #
    _sc_v = nc.named_scope("vphase"); _sc_v.__enter__()
    vw = []
    for kk in range(KK):
        # tag shared with the proj slabs (same shape/dtype, disjoint lifetime)
        wt = wpool.tile([128, 2, C], F8, tag=f"wmid{kk}", name="wv")
        nc.sync.dma_start(out=wt[:], in_=p["vw8"][kk])
        vw.append(wt)
    nc.vector.memset(v_pre[:, :, :, DH : DH + 1], 1.0)
    for tcn in range(T // 128):
        accs = [ps1.tile([128, 512], F32, tag="mm", bufs=4, name=f"vacc{tcn}_{i}") for i in range(2)]
        for kk in range(KK):
            for vf in range(2):
                nc.tensor.matmul(
                    accs[vf][:], h1[:, ds(2 * kk, 2), ds(128 * tcn, 128)],
                    vw[kk][:, :, ds(512 * vf, 512)],
                    start=(kk == 0), stop=False, skip_group_check=True,
                    perf_mode=DR,
                )
        for vf in range(2):
            nc.tensor.matmul(
                accs[vf][:], ones1_128[:], vb2[:, ds(512 * vf, 512)],
                start=False, stop=True, skip_group_check=True,
            )
            nc.vector.tensor_scalar_mul(
                out=v_pre[:, tcn, ds(8 * vf, 8), 0:DH],
                in0=accs[vf][:].rearrange("p (h d) -> p h d", d=DH),
                scalar1=1.0 / WS,
            )
    _sc_v.__exit__(None, None, None)
    # V all-gather split in halves (overlaps Q compute + attention start)
    with nc.named_scope("agV"):
        for h in range(2):
            for pr in range(4 * h, 4 * h + 4):
                nc.sync.dma_start(
                    out=v_in2[pr].rearrange("(tc p) h d -> p tc h d", p=128),
                    in_=v_pre[:, :, ds(2 * pr, 2), :],
                )
            nc.gpsimd.collective_compute(
                "AllGather", mybir.AluOpType.bypass,
                replica_groups=REPLICA_GROUPS,
                ins=[v_in2[ds(4 * h, 4)].opt()], outs=[v_gh[h].opt()],
            )

    # ---- Q tiles last (overlap the gathers) ------------------------------- #
    _sc_q = nc.named_scope("qtiles"); _sc_q.__enter__()
    qk_block(list(range(0, 8)), True)
    rqn = make_rnorm(ssq_q, True)
    rnorm_apply(rqn, rq_d, qe, ps1)
    _sc_q.__exit__(None, None, None)

    ps1.release()

    # ============================ attention ================================ #
    ps2 = tc.alloc_tile_pool(name="ps2", bufs=1, space="PSUM")
    vpool = tc.alloc_tile_pool(name="vpool", bufs=4)
    ptpool = tc.alloc_tile_pool(name="ptpool", bufs=3)

    den = stats.tile([16, T], F32, tag="den")     # softmax denominators
    KT = L // 128  # 16 key tiles
    _sc_at = nc.named_scope("attn"); _sc_at.__enter__()
    for pr in range(8):  # head pairs
        vt = vpool.tile([128, GROUP, KT // 4, 2, DH + 1], BF16, tag="v")
        vsrc = v_gh.rearrange(
            "s r y (kt p) h d -> s y r p kt (h d)", p=128
        )[pr // 4, pr % 4]
        for rr in range(GROUP):
            nc.sync.dma_start(
                out=vt.rearrange("p r k h d -> r p k (h d)")[rr],
                in_=vsrc[rr],
            )
        o_ps = [ps2.tile([DH + 1, T], F32, tag="o", bufs=4, name=f"o{pr}_{i}") for i in range(2)]
        for kt in range(KT):
            r, ktc = divmod(kt, 4)
            sp = ps2.tile([128, 2 * T], F32, tag="sc", bufs=2)
            for j in range(2):
                nc.tensor.matmul(
                    sp[:, ds(T * j, T)],
                    kfull[ds(64 * j, 64), pr, r, ds(128 * ktc, 128)],
                    qe[ds(64 * j, 64), pr, :],
                    start=True, stop=True,
                )
            pt = ptpool.tile([128, 2 * T], BF16, tag="pt")
            nc.scalar.activation(out=pt[:], in_=sp[:], func=AF.Exp)
            # bias multiply split ~2:1 across DVE and GpSimd (GpSimd measures
            # ~2x slower per element, so it gets the smaller share)
            for j in range(2):
                eng = nc.gpsimd if (2 * kt + j) % 3 == 2 else nc.vector
                eng.tensor_tensor(out=pt[:, ds(T * j, T)],
                                  in0=pt[:, ds(T * j, T)],
                                  in1=eb[:, kt, :], op=mybir.AluOpType.mult)
            for j in range(2):
                nc.tensor.matmul(
                    o_ps[j][:], vt[:, r, ktc, j, :], pt[:, ds(T * j, T)],
                    start=(kt == 0), stop=(kt == KT - 1), skip_group_check=True,
                )
        for j in range(2):
            # park the unscaled numerator + stage the denominator row
            # (engine APs need 32-aligned partition bases, so bounce the
            #  denominator rows through DRAM to collect them on 16 partitions)
            nc.vector.tensor_copy(out=anum[ds(64 * j, 64), pr, :],
                                  in_=o_ps[j][0:DH, :])
            dstage = work.tile([1, T], F32, tag="dstage", name="dstage")
            nc.vector.tensor_copy(out=dstage[:], in_=o_ps[j][ds(DH, 1), :])
            nc.sync.dma_start(out=den_d[ds(2 * pr + j, 1), :], in_=dstage[:])
    nc.sync.dma_start(out=den[:], in_=den_d[:])

    # batched softmax normalization: one reciprocal, selector-matmul bcast
    recs = stats.tile([16, T], BF16, tag="recs")
    with nc.allow_low_precision(reason="softmax denom recip in bf16 is fine at 2e-2 tol"):
        nc.vector.reciprocal(out=recs[:], in_=den[:])
    for pr in range(8):
        bcp = ps2.tile([128, T], F32, tag="sc", bufs=2, name="bcp")
        nc.tensor.matmul(bcp[:], hselT[:, pr, :], recs[:], start=True, stop=True)
        nc.vector.tensor_tensor(
            out=attn[:, pr, :], in0=anum[:, pr, :], in1=bcp[:],
            op=mybir.AluOpType.mult,
        )

    _sc_at.__exit__(None, None, None)
    ptpool.release()
    vpool.release()
    ps2.release()

    # ============================ proj + FFN =============================== #
    ps3 = tc.alloc_tile_pool(name="ps3", bufs=1, space="PSUM")

    def dense8(wname, nkk, Mtiles, rhs, consume, wtag, resident, mm_bufs=8,
               mgrp=4):
        """fp8 DoubleRow dense layer: out[m] += w8[kk][:,:,m].T2 @ rhs pairs."""
        if resident:
            slabs = []
            for kk in range(nkk):
                wt = wpool.tile([128, 2, 128 * Mtiles], F8,
                                tag=f"wmid{kk}", name=wtag)
                nc.sync.dma_start(out=wt[:], in_=p[wname][kk])
                slabs.append(wt)
            for mg in range(0, Mtiles, mgrp):
                nsub = min(mgrp, Mtiles - mg)
                accs = [ps3.tile([128, T], F32, tag="mm", bufs=mm_bufs,
                                 name=f"d{wtag}{mg}_{i}") for i in range(nsub)]
                for kk in range(nkk):
                    for i in range(nsub):
                        nc.tensor.matmul(
                            accs[i][:],
                            slabs[kk][:, :, ds(128 * (mg + i), 128)],
                            rhs[:, ds(2 * kk, 2), :],
                            start=(kk == 0), stop=(kk == nkk - 1), perf_mode=DR,
                        )
                for i in range(nsub):
                    consume(mg + i, accs[i])
        else:
            # stream slabs: single m-group covering all Mtiles (kk outer)
            accs = [ps3.tile([128, T], F32, tag="mm", bufs=mm_bufs,
                             name=f"d{wtag}_{i}") for i in range(Mtiles)]
            for kk in range(nkk):
                wt = wpool.tile([128, 2, 128 * Mtiles], F8, tag=wtag, bufs=3,
                                name=wtag)
                nc.sync.dma_start(out=wt[:], in_=p[wname][kk])
                for i in range(Mtiles):
                    nc.tensor.matmul(
                        accs[i][:], wt[:, :, ds(128 * i, 128)],
                        rhs[:, ds(2 * kk, 2), :],
                        start=(kk == 0), stop=(kk == nkk - 1), perf_mode=DR,
                    )
            for i in range(Mtiles):
                consume(i, accs[i])

    # proj -> x2 = xb' + proj_out * g1   (proj_b*g1 was pre-folded into xb)
    def proj_consume(m, acc):
        nc.vector.scalar_tensor_tensor(
            out=x2[:, m, :], in0=acc[:], scalar=modg[:, ds(m, 1)],
            in1=xb[:, m, :],
            op0=mybir.AluOpType.mult, op1=mybir.AluOpType.add,
        )

    with nc.named_scope("proj"):
        dense8("projw8", KK, CT, attn, proj_consume, "wpj", resident=True)

    with nc.named_scope("ln2"):
        layernorm(x2, h2, 24, 40, ps3, 8)  # s2 cols 24..31, sh2 cols 40..47
    # fold fc2_b*g2 into the residual now so fc2_consume is a single op
    for t in range(CT):
        nc.vector.tensor_scalar_add(out=x2[:, t, :], in0=x2[:, t, :],
                                    scalar1=fbg2[:, ds(t, 1)])

    def fc1_consume(m, acc):
        if not sim_gelu:
            nc.scalar.activation(
                out=gact[:, m, :], in_=acc[:], func=AF.Gelu_apprx_tanh,
                bias=f1b[:, ds(m, 1)], scale=1.0 / WS,
            )
            return
        # simulator fallback: explicit tanh-approx gelu
        xs = work.tile([128, T], F32, tag="d1", name="xs")
        nc.scalar.activation(out=xs[:], in_=acc[:], func=AF.Identity,
                             bias=f1b[:, ds(m, 1)], scale=1.0 / WS)
        t1 = work.tile([128, T], F32, tag="gsim", name="t1")
        nc.vector.tensor_tensor(out=t1[:], in0=xs[:], in1=xs[:],
                                op=mybir.AluOpType.mult)
        nc.vector.tensor_tensor(out=t1[:], in0=t1[:], in1=xs[:],
                                op=mybir.AluOpType.mult)
        nc.vector.scalar_tensor_tensor(
            out=t1[:], in0=t1[:], scalar=0.044715, in1=xs[:],
            op0=mybir.AluOpType.mult, op1=mybir.AluOpType.add,
        )
        nc.scalar.activation(out=t1[:], in_=t1[:], func=AF.Tanh,
                             scale=0.7978845608028654)
        nc.vector.tensor_scalar(
            out=t1[:], in0=t1[:], scalar1=0.5, scalar2=0.5,
            op0=mybir.AluOpType.mult, op1=mybir.AluOpType.add,
        )
        nc.vector.tensor_tensor(out=gact[:, m, :], in0=t1[:], in1=xs[:],
                                op=mybir.AluOpType.mult)

    with nc.named_scope("fc1"):
        # two half-layers of 16 m-tiles each; 4 resident slabs per half
        for half in range(2):
            slabs = []
            for kk in range(KK):
                wt = wpool.tile([128, 2, 2048], F8, tag=f"wbig{kk}", name="wf1")
                nc.sync.dma_start(
                    out=wt[:], in_=p["fc1w8"][kk][:, :, ds(2048 * half, 2048)]
                )
                slabs.append(wt)
            for mg in range(0, 16, 4):
                accs = [ps3.tile([128, T], F32, tag="mm", bufs=8,
                                 name=f"df1{half}_{mg}_{i}") for i in range(4)]
                for kk in range(KK):
                    for i in range(4):
                        nc.tensor.matmul(
                            accs[i][:],
                            slabs[kk][:, :, ds(128 * (mg + i), 128)],
                            h2[:, ds(2 * kk, 2), :],
                            start=(kk == 0), stop=(kk == KK - 1), perf_mode=DR,
                        )
                for i in range(4):
                    fc1_consume(16 * half + mg + i, accs[i])

    def fc2_consume(m, acc):
        d1 = work.tile([128, T], F32, tag="d1")
        nc.vector.scalar_tensor_tensor(
            out=d1[:], in0=acc[:], scalar=modg[:, ds(8 + m, 1)], in1=x2[:, m, :],
            op0=mybir.AluOpType.mult, op1=mybir.AluOpType.add,
        )
        nc.sync.dma_start(
            out=out_d.rearrange("(t p) q -> t p q", p=128)[m], in_=d1[:]
        )

    with nc.named_scope("fc2"):
        dense8("fc2w8", DFF // 256, CT, gact, fc2_consume, "wf2",
               resident=False, mm_bufs=8)

    ps3.release()
    for pool in (dram, wpool, stats, work, persist, const):
        pool.release()


# --------------------------------------------------------------------------- #
# host side: shard, run, gather
# --------------------------------------------------------------------------- #

def _shard(inputs):
    bf = ml_dtypes.bfloat16
    f8 = ml_dtypes.float8_e4m3
    x = np.asarray(inputs["x"], np.float32)
    cond = np.asarray(inputs["cond_BD"], np.float32)
    bias = np.asarray(inputs["attn_bias"], np.float32)[0, 0]  # [L, L]
    qkv_w = np.asarray(inputs["qkv_w"], np.float32)
    q_bias = np.asarray(inputs["q_bias"], np.float32)
    v_bias = np.asarray(inputs["v_bias"], np.float32)
    scale_mul = np.asarray(inputs["scale_mul"], np.float32).reshape(H)
    proj_w = np.asarray(inputs["proj_w"], np.float32)
    proj_b = np.asarray(inputs["proj_b"], np.float32)
    fc1_w = np.asarray(inputs["fc1_w"], np.float32)
    fc1_b = np.asarray(inputs["fc1_b"], np.float32)
    fc2_w = np.asarray(inputs["fc2_w"], np.float32)
    fc2_b = np.asarray(inputs["fc2_b"], np.float32)
    ada_w = np.asarray(inputs["ada_w"], np.float32)
    ada_b = np.asarray(inputs["ada_b"], np.float32)

    hsel = np.zeros((128, CT, 16), np.float32)
    for t in range(CT):
        hsel[:64, t, 2 * t] = 1.0
        hsel[64:, t, 2 * t + 1] = 1.0
    hselT = np.ascontiguousarray(hsel.transpose(2, 1, 0))  # [16, CT, 128]
    pairsel_np = np.zeros((2, 128), np.float32)
    pairsel_np[0, :64] = 1.0
    pairsel_np[1, 64:] = 1.0

    def pair_w(wT, M):
        # [C_in, M] -> [C_in//256, 128, 2, M] fp8 DoubleRow slabs, scaled x WS
        nkk = wT.shape[0] // 256
        w = np.clip(wT * WS, -240.0, 240.0).reshape(nkk, 2, 128, M)
        return np.ascontiguousarray(w.transpose(0, 2, 1, 3)).astype(f8)

    qkvT = qkv_w.T  # [C, 3C]
    shared = {
        "adawT": np.ascontiguousarray(
            ada_w.T.reshape(CT, 128, 6, 1024).transpose(2, 0, 1, 3)
        ).astype(bf),
        "qkw8": pair_w(qkvT[:, : 2 * C], 2 * C),
        "vw8": pair_w(qkvT[:, 2 * C :], C),
        "projw8": pair_w(proj_w.T, C),
        "fc1w8": pair_w(fc1_w.T, DFF),
        "fc2w8": pair_w(fc2_w.T, C),

        "adab48": np.ascontiguousarray(ada_b.reshape(48, 128).T),
        "qb8": np.ascontiguousarray(q_bias.reshape(CT, 128).T),
        "vb2": (v_bias.reshape(1, C) * WS).astype(bf),
        "pb8": np.ascontiguousarray(proj_b.reshape(CT, 128).T),
        "f1b": np.ascontiguousarray(fc1_b.reshape(DFF // 128, 128).T),
        "f2b": np.ascontiguousarray(fc2_b.reshape(CT, 128).T),
        "smv": scale_mul.reshape(16, 1).copy(),
        "ones128": np.ones((128, 128), np.float32).astype(bf),
        "hsel": hsel.astype(bf),
        "hselT": hselT.astype(bf),
        "ones1_128": np.ones((1, 128), np.float32).astype(bf),
        "pairsel": pairsel_np.astype(bf),
        "eye48": np.eye(48, dtype=np.float32),
    }

    in_maps = []
    for core in range(NCORES):
        g, r = divmod(core, GROUP)
        qs = slice(T * r, T * (r + 1))
        m = dict(shared)
        m["xb"] = np.ascontiguousarray(
            x[g, qs].T.reshape(CT, 128, T)
        ).astype(bf)
        m["cond8"] = np.ascontiguousarray(cond[g].reshape(8, 128).T)
        m["biasT"] = np.ascontiguousarray(
            bias[qs].T.reshape(16, 128, T)
        ).astype(bf)

        in_maps.append(m)
    return in_maps


def kernel(**inputs):
    if "nc" not in _CACHE:
        _CACHE["nc"] = _build()
    nc = _CACHE["nc"]
    in_maps = _shard(inputs)
    try:
        res = bass_utils.run_bass_kernel_spmd(
            nc, in_maps, core_ids=list(range(NCORES))
        )
    except Exception:
        # transient device-state hiccup (seen after profiled runs); retry once
        res = bass_utils.run_bass_kernel_spmd(
            nc, in_maps, core_ids=list(range(NCORES))
        )
    out = np.empty((B, L, C), np.float32)
    for core in range(NCORES):
        g, r = divmod(core, GROUP)
        out[g, T * r : T * (r + 1)] = res.results[core]["out"].T
    return out


# revision 61
# speedup vs baseline: 1.0565x; 1.0565x over previous
"""AdaLN self-attention block (B=2, L=2048, C=1024, H=16, DFF=4096) on 8 TRN2 cores.

Sharding: DP=2 over batch (cores 0-3 -> batch 0, cores 4-7 -> batch 1),
sequence-parallel 4-way within each group (512 query tokens per core).
Each core holds full weights, computes q/k/v for its own 512 tokens,
all-gathers normalized K and V (with an appended ones column for the softmax
denominator) within its 4-core group, runs full attention for its queries,
then proj + FFN locally on its token slice. Host concatenates the slices.

Dense matmuls (qkv / v / proj / fc1 / fc2) run in fp8e4m3 with DoubleRow
perf mode (256-deep contraction per pass, ~1.7x PE throughput). Weights are
scaled x64 on the host so they sit in e4m3's normal range; the descale by
1/64 is folded into each consumer's existing scalar op. Activations feeding
those matmuls (h1, h2, attn, gelu) are written as fp8 directly by their
producing ops. Attention QK / AV and the residual stream stay bf16.

Everything on-chip is feature-major ([C, tokens]); the host pre-transposes
activations/weights so no on-device transposes are needed (except a tiny
48x128 one for the adaLN modulation vector).
"""

import os
import sys

for _p in ("/opt/trn_rl_repo", os.path.expanduser("~/.axon_site/_ro/trn_rl_repo")):
    if os.path.isdir(_p) and _p not in sys.path:
        sys.path.insert(0, _p)

import numpy as np
import ml_dtypes

import concourse.bass as bass
import concourse.tile as tile
from concourse import mybir
from concourse.bass import ds, ts
from concourse import bass_utils

BF16 = mybir.dt.bfloat16
F32 = mybir.dt.float32
F8 = mybir.dt.float8e4
AF = mybir.ActivationFunctionType
DR = mybir.MatmulPerfMode.DoubleRow

B, L, C, H, DH, DFF, D = 2, 2048, 1024, 16, 64, 4096, 1024
NCORES = 8
GROUP = 4          # cores per batch group
T = L // GROUP     # 512 query tokens per core
CT = C // 128      # 8 feature tiles
KK = C // 256      # 4 DoubleRow contraction slabs over C
ADA_SLICE = 6 * C // GROUP  # 1536 adaLN outputs per core
EPS = 1e-6
MAX_SCALE_MUL = float(np.log(100.0))
WS = 64.0          # fp8 weight scale
REPLICA_GROUPS = [[0, 1, 2, 3], [4, 5, 6, 7]]

_CACHE = {}


# --------------------------------------------------------------------------- #
# graph construction
# --------------------------------------------------------------------------- #

def _build(sim_gelu=False, split_waits=True):
    nc = bass.Bass(
        "TRN2", target_bir_lowering=False, debug=False, num_devices=NCORES
    )

    def inp(name, shape, dt):
        return nc.dram_tensor(name, shape, dt, kind="ExternalInput").ap()

    p = {
        "xb": inp("xb", [CT, 128, T], BF16),      # x^T slice, tiled, bf16
        "cond8": inp("cond8", [128, 8], F32),     # cond feature-major
        "biasT": inp("biasT", [16, 128, T], BF16),  # exp-bias source, tiled
        "qkw8": inp("qkw8", [KK, 128, 2, 2 * C], F8),
        "vw8": inp("vw8", [KK, 128, 2, C], F8),
        "projw8": inp("projw8", [KK, 128, 2, C], F8),
        "fc1w8": inp("fc1w8", [KK, 128, 2, DFF], F8),
        "fc2w8": inp("fc2w8", [DFF // 256, 128, 2, C], F8),
        "adawT": inp("adawT", [3, CT, 128, 2048], BF16),
        "adab48": inp("adab48", [128, 48], F32),
        "qb8": inp("qb8", [128, CT], F32),
        "vb2": inp("vb2", [1, C], BF16),          # host-scaled x WS
        "pb8": inp("pb8", [128, CT], F32),
        "f1b": inp("f1b", [128, DFF // 128], F32),
        "f2b": inp("f2b", [128, CT], F32),
        "smv": inp("smv", [16, 1], F32),
        "ones128": inp("ones128", [128, 128], BF16),
        "hsel": inp("hsel", [128, CT, 16], BF16),
        "hselT": inp("hselT", [16, CT, 128], BF16),
        "ones1_128": inp("ones1_128", [1, 128], BF16),
        "pairsel": inp("pairsel", [2, 128], BF16),
        "eye48": inp("eye48", [48, 48], F32),
    }
    out = nc.dram_tensor("out", [C, T], F32, kind="ExternalOutput").ap()

    with tile.TileContext(nc) as tc:
        _emit(nc, tc, p, out, sim_gelu)
    if split_waits:
        _split_waits(nc)
    return nc


_SPLIT_TYPES = {
    "InstTensorTensor", "InstTensorScalarPtr", "InstReciprocal",
    "InstTensorCopy", "InstActivation", "InstTensorReduce", "InstMemset",
    "InstMatmult", "InstLdweights", "InstCopyPredicated", "InstBnStats",
    "InstBnAggr", "InstStreamTranspose", "InstDMACopy", "InstDrain",
    "InstCollectiveCompute",
}


def _split_waits(nc, max_waits=1):
    """Walrus TPB codegen rejects >1 sync-wait on compute instructions;
    hoist extras onto standalone EventSemaphore waits on the same engine."""
    for bb in nc.main_func.blocks:
        new = []
        changed = False
        for ins in bb.instructions:
            si = getattr(ins, "sync_info", None)
            if (
                si is not None
                and si.on_wait
                and len(si.on_wait) > max_waits
                and type(ins).__name__ in _SPLIT_TYPES
            ):
                waits = list(si.on_wait)
                for i, w in enumerate(waits[:-max_waits]):
                    ws = mybir.InstEventSemaphore(
                        name=f"{ins.name}_w{i}", ins=[], outs=[]
                    )
                    ws.engine = ins.engine
                    ws.sync_info = mybir.SyncInfo(on_wait=[w], on_update=[])
                    new.append(ws)
                ins.sync_info = mybir.SyncInfo(
                    on_wait=waits[-max_waits:], on_update=list(si.on_update)
                )
                changed = True
            new.append(ins)
        if changed:
            bb.instructions = new


def _emit(nc, tc, p, out_d, sim_gelu=False):

    # ---- persistent SBUF pools -------------------------------------------- #
    const = tc.alloc_tile_pool(name="const", bufs=1)
    persist = tc.alloc_tile_pool(name="persist", bufs=1)
    work = tc.alloc_tile_pool(name="work", bufs=4)
    stats = tc.alloc_tile_pool(name="stats", bufs=1)
    wpool = tc.alloc_tile_pool(name="wpool", bufs=1)
    dram = tc.alloc_tile_pool(name="dram", bufs=1, space="DRAM")

    # ---- constants / small inputs to SBUF --------------------------------- #
    def load_const(name, shape, dt):
        t = const.tile(shape, dt, tag=name, name=name)
        nc.sync.dma_start(out=t[:], in_=p[name])
        return t

    # ada path inputs first: its collective is the first serialization point
    cond8 = load_const("cond8", [128, 8], F32)
    adab48 = load_const("adab48", [128, 48], F32)
    eye48 = load_const("eye48", [48, 48], F32)
    ones128 = load_const("ones128", [128, 128], BF16)
    hsel = load_const("hsel", [128, CT, 16], BF16)
    hselT = load_const("hselT", [16, CT, 128], BF16)
    ones1_128 = load_const("ones1_128", [1, 128], BF16)
    pairsel = load_const("pairsel", [2, 128], BF16)
    qb8 = load_const("qb8", [128, CT], F32)
    vb2 = load_const("vb2", [1, C], BF16)
    pb8 = load_const("pb8", [128, CT], F32)
    f1b = load_const("f1b", [128, DFF // 128], F32)
    f2b = load_const("f2b", [128, CT], F32)
    smv_in = load_const("smv", [16, 1], F32)

    # ---- DRAM bounce buffers ---------------------------------------------- #
    ada_l = dram.tile([1, 6 * C], F32, tag="ada_l")
    k_in = dram.tile([C, T], BF16, tag="k_in")
    k_gh = dram.tile([2, GROUP, 4, 128, T], BF16, tag="k_gh")
    v_in = dram.tile([T, H, DH + 1], BF16, tag="v_in")
    v_g = dram.tile([GROUP, T, H, DH + 1], BF16, tag="v_g")
    rq_d = dram.tile([16, T], BF16, tag="rq_d")
    rk_d = dram.tile([16, T], BF16, tag="rk_d")
    den_d = dram.tile([16, T], F32, tag="den_d")

    # ---- adaLN: silu(cond), then full 6C ada vector computed redundantly -- #
    sig = work.tile([128, 8], F32, tag="w8")
    nc.scalar.activation(out=sig[:], in_=cond8[:], func=AF.Exp, scale=-1.0)
    nc.vector.tensor_scalar_add(out=sig[:], in0=sig[:], scalar1=1.0)
    nc.vector.reciprocal(out=sig[:], in_=sig[:])
    silu = work.tile([128, 8], BF16, tag="w8b")
    nc.vector.tensor_tensor(
        out=silu[:], in0=sig[:], in1=cond8[:], op=mybir.AluOpType.mult
    )

    # Every core computes the full 6C adaLN vector redundantly: a collective
    # here costs ~55us of trigger latency, the redundant matmuls only ~25us.
    ps0 = tc.alloc_tile_pool(name="ps0", bufs=1, space="PSUM")
    _sc_ada = nc.named_scope("ada"); _sc_ada.__enter__()
    for ngp in range(3):
        aps = [ps0.tile([1, 1024], F32, tag="ada", bufs=2, name=f"aps{ngp}_{i}")
               for i in range(2)]
        for k in range(CT):
            wt = wpool.tile([128, 2, 1024], BF16, tag="wada", bufs=2,
                            name="wada")
            nc.sync.dma_start(
                out=wt.rearrange("p i c -> p (i c)"),
                in_=p["adawT"][ngp, k],
            )
            for i in range(2):
                for s in range(2):
                    nc.tensor.matmul(
                        aps[i][0:1, ds(512 * s, 512)], silu[:, ds(k, 1)],
                        wt[:, i, ds(512 * s, 512)],
                        start=(k == 0), stop=(k == CT - 1),
                        skip_group_check=True,
                    )
        for i in range(2):
            aw = work.tile([1, 1024], F32, tag="w1x512", bufs=2, name="aw")
            nc.vector.tensor_copy(out=aw[:], in_=aps[i][:])
            nc.sync.dma_start(
                out=ada_l[0, ds(2048 * ngp + 1024 * i, 1024)], in_=aw[:]
            )
    ps0.release()

    # ============================ phase 1 PSUM ============================= #
    ps1 = tc.alloc_tile_pool(name="ps1", bufs=1, space="PSUM")

    # load [48,128] token-major, transpose on PE -> mod [128, 48]
    mod = persist.tile([128, 48], F32, tag="mod")
    ada_tm = work.tile([48, 128], F32, tag="ada_tm")
    nc.sync.dma_start(out=ada_tm[:], in_=ada_l.rearrange("g n -> (g n)").rearrange("(j p) -> j p", p=128))
    modps = ps1.tile([128, 48], F32, tag="sm", bufs=2)
    nc.tensor.transpose(modps[:], ada_tm[:], eye48[:])
    nc.vector.tensor_tensor(out=mod[:], in0=modps[:], in1=adab48[:],
                            op=mybir.AluOpType.add)
    # s1, s2 chunks get +1
    nc.vector.tensor_scalar_add(out=mod[:, 16:32], in0=mod[:, 16:32], scalar1=1.0)
    # descaled copies of g1 / g2 columns for fp8 PSUM consumers
    modg = stats.tile([128, 16], F32, tag="modg")
    nc.vector.tensor_scalar_mul(out=modg[:], in0=mod[:, 0:16], scalar1=1.0 / WS)
    pbg1 = stats.tile([128, CT], F32, tag="pbg1")
    nc.vector.tensor_tensor(out=pbg1[:], in0=pb8[:], in1=mod[:, 0:8],
                            op=mybir.AluOpType.mult)
    fbg2 = stats.tile([128, CT], F32, tag="fbg2")
    nc.vector.tensor_tensor(out=fbg2[:], in0=f2b[:], in1=mod[:, 8:16],
                            op=mybir.AluOpType.mult)
    _sc_ada.__exit__(None, None, None)

    # ---- persistent activations ------------------------------------------- #
    xb = persist.tile([128, CT, T], BF16, tag="big_d")       # x^T bf16
    nc.sync.dma_start(out=xb[:], in_=p["xb"].rearrange("t p q -> p t q"))

    h1 = persist.tile([128, CT, T], F8, tag="big_a")         # LN1-mod, fp8
    qe = persist.tile([128, CT, T], BF16, tag="big_b")       # q (later normed)
    ke = persist.tile([128, CT, T], BF16, tag="big_c")       # k (later normed)
    v_pre = persist.tile([128, T // 128, H, DH + 1], BF16, tag="vpre")
    eb = persist.tile([128, L // 128, T], BF16, tag="eb")    # exp(bias^T)
    attn = persist.tile([128, CT, T], F8, tag="big_a")       # fp8 probs@V
    anum = persist.tile([128, CT, T], BF16, tag="vpre")      # unscaled attn out
    x2 = persist.tile([128, CT, T], BF16, tag="big_c")
    h2 = persist.tile([128, CT, T], F8, tag="big_b")
    gact = persist.tile([128, DFF // 128, T], F8, tag="big_d")
    vfull = persist.tile([128, GROUP, T // 128, H, DH + 1], BF16, tag="vfull")

    # ---- expbias (independent; emitted early so it overlaps) -------------- #
    nc.sync.dma_start(out=eb[:], in_=p["biasT"].rearrange("t p q -> p t q"))
    for i in range(4):
        nc.scalar.activation(
            out=eb[:, ds(4 * i, 4), :], in_=eb[:, ds(4 * i, 4), :], func=AF.Exp
        )

    # ---- scale_mul -> smv = exp(min(scale_mul, log 100)) ------------------ #
    eps128 = const.tile([128, 1], F32, tag="eps128")
    nc.vector.memset(eps128[:], EPS)
    smv = stats.tile([16, 1], F32, tag="smv")
    nc.vector.tensor_scalar_min(out=smv[:], in0=smv_in[:], scalar1=MAX_SCALE_MUL)
    nc.scalar.activation(out=smv[:], in_=smv[:], func=AF.Exp)

    # ---- layernorm helper (feature-major, partition sums via ones matmul) - #
    def layernorm(src, dst, s_col, sh_col, psp, mm_bufs):
        s1 = psp.tile([128, T], F32, tag="mm", bufs=mm_bufs)
        s2 = psp.tile([128, T], F32, tag="mm", bufs=mm_bufs)
        for t in range(CT):
            sq = work.tile([128, T], BF16, tag="sq")
            nc.vector.tensor_tensor(
                out=sq[:], in0=src[:, t, :], in1=src[:, t, :], op=mybir.AluOpType.mult
            )
            nc.tensor.matmul(s1[:], ones128[:], src[:, t, :],
                             start=(t == 0), stop=(t == CT - 1), skip_group_check=True)
            nc.tensor.matmul(s2[:], ones128[:], sq[:],
                             start=(t == 0), stop=(t == CT - 1), skip_group_check=True)
        meanb = stats.tile([128, T], F32, tag="meanb")
        nc.vector.tensor_scalar_mul(out=meanb[:], in0=s1[:], scalar1=1.0 / C)
        m2 = stats.tile([128, T], F32, tag="m2")
        nc.vector.tensor_tensor(out=m2[:], in0=meanb[:], in1=meanb[:],
                                op=mybir.AluOpType.mult)
        varb = stats.tile([128, T], F32, tag="varb")
        nc.vector.scalar_tensor_tensor(
            out=varb[:], in0=s2[:], scalar=1.0 / C, in1=m2[:],
            op0=mybir.AluOpType.mult, op1=mybir.AluOpType.subtract,
        )
        # rstd = exp(-0.5 * ln(var + eps))   (stays in the exp/ln table set)
        nc.scalar.activation(out=varb[:], in_=varb[:], func=AF.Ln, bias=eps128[:])
        rstdb = stats.tile([128, T], F32, tag="rstdb")
        nc.scalar.activation(out=rstdb[:], in_=varb[:], func=AF.Exp, scale=-0.5)
        for t in range(CT):
            d1 = work.tile([128, T], F32, tag="d1")
            nc.vector.tensor_tensor(out=d1[:], in0=src[:, t, :], in1=meanb[:],
                                    op=mybir.AluOpType.subtract)
            nc.vector.tensor_tensor(out=d1[:], in0=d1[:], in1=rstdb[:],
                                    op=mybir.AluOpType.mult)
            nc.vector.tensor_scalar(
                out=dst[:, t, :], in0=d1[:],
                scalar1=mod[:, ds(s_col + t, 1)], scalar2=mod[:, ds(sh_col + t, 1)],
                op0=mybir.AluOpType.mult, op1=mybir.AluOpType.add,
            )

    with nc.named_scope("ln1"):
        layernorm(xb, h1, 16, 32, ps1, 4)  # s1 cols 16..23, sh1 cols 32..39
    # fold proj_b*g1 into the residual now so proj_consume is a single op
    for t in range(CT):
        nc.vector.tensor_scalar_add(out=xb[:, t, :], in0=xb[:, t, :],
                                    scalar1=pbg1[:, ds(t, 1)])

    # ---- qkv weights: 4 resident fp8 slabs -------------------------------- #
    qkw = []
    for kk in range(KK):
        # tag shared with the fc1 slabs (same shape/dtype, disjoint lifetime)
        wt = wpool.tile([128, 2, 2 * C], F8, tag=f"wbig{kk}", name="wqk")
        nc.sync.dma_start(out=wt[:], in_=p["qkw8"][kk])
        qkw.append(wt)

    # ---- qkv: K first (so its all-gather overlaps V and Q compute) ------- #
    _sc_qkv = nc.named_scope("qkv"); _sc_qkv.__enter__()
    ssq_q = ps1.tile([16, T], F32, tag="ss", bufs=2)
    ssq_k = ps1.tile([16, T], F32, tag="ss", bufs=2)

    def qk_block(ms, is_q):
        # ms: global m-tile indices into the 2C q/k output (0..7 q, 8..15 k)
        for mg in range(0, len(ms), 4):
            sub = ms[mg:mg + 4]
            accs = [ps1.tile([128, T], F32, tag="mm", bufs=4, name=f"qk{m}")
                    for m in sub]
            for kk in range(KK):
                for i, m in enumerate(sub):
                    nc.tensor.matmul(
                        accs[i][:], qkw[kk][:, :, ds(128 * m, 128)],
                        h1[:, ds(2 * kk, 2), :],
                        start=(kk == 0), stop=(kk == KK - 1), perf_mode=DR,
                    )
            for i, m in enumerate(sub):
                acc = accs[i]
                if is_q:
                    dst = qe[:, m, :]
                    nc.vector.tensor_scalar(
                        out=dst, in0=acc[:], scalar1=1.0 / WS,
                        scalar2=qb8[:, ds(m, 1)],
                        op0=mybir.AluOpType.mult, op1=mybir.AluOpType.add,
                    )
                else:
                    dst = ke[:, m - 8, :]
                    nc.vector.tensor_scalar_mul(out=dst, in0=acc[:],
                                                scalar1=1.0 / WS)
                sq = work.tile([128, T], BF16, tag="sq")
                nc.vector.tensor_tensor(out=sq[:], in0=dst, in1=dst,
                                        op=mybir.AluOpType.mult)
                tgt = ssq_q if is_q else ssq_k
                tm = m % 8
                nc.tensor.matmul(tgt[:], hsel[:, tm, :], sq[:],
                                 start=(tm == 0), stop=(tm == 7),
                                 skip_group_check=True)

    def make_rnorm(ssq, with_sm):
        r = stats.tile([16, T], F32, tag="rn_f")
        nc.vector.tensor_scalar_max(out=r[:], in0=ssq[:], scalar1=1e-24)
        nc.scalar.activation(out=r[:], in_=r[:], func=AF.Ln)
        rb = stats.tile([16, T], BF16, tag="rn_bq" if with_sm else "rn_bk", name="rb")
        nc.scalar.activation(out=rb[:], in_=r[:], func=AF.Exp, scale=-0.5)
        if with_sm:
            nc.vector.tensor_scalar_mul(out=rb[:], in0=rb[:], scalar1=smv[:])
        return rb

    def rnorm_apply(rb, rd_bounce, dst, psp):
        # partition remap [16,T] -> [2,8,T] via a DRAM roundtrip, then a
        # K=2 pairsel matmul broadcasts each head row over its 64 partitions
        nc.sync.dma_start(out=rd_bounce[:], in_=rb[:])
        rn2 = work.tile([2, 8, T], BF16, tag="rn2", bufs=1, name="rn2")
        nc.sync.dma_start(out=rn2[:],
                          in_=rd_bounce.rearrange("(t j) q -> j t q", j=2))
        for t in range(CT):
            bc = psp.tile([128, T], F32, tag="sm", bufs=2, name="bcn")
            nc.tensor.matmul(bc[:], pairsel[:], rn2[:, t, :], start=True, stop=True)
            nc.vector.tensor_tensor(out=dst[:, t, :], in0=dst[:, t, :], in1=bc[:],
                                    op=mybir.AluOpType.mult)

    qk_block(list(range(8, 16)), False)  # K tiles
    rkn = make_rnorm(ssq_k, False)
    rnorm_apply(rkn, rk_d, ke, ps1)
    _sc_qkv.__exit__(None, None, None)

    # ---- K all-gather, split in halves so attention can start early ------- #
    kin_t = k_in.rearrange("(t p) q -> t p q", p=128)
    with nc.named_scope("agK"):
        for h in range(2):
            for t in range(4 * h, 4 * h + 4):
                nc.sync.dma_start(out=kin_t[t], in_=ke[:, t, :])
            nc.gpsimd.collective_compute(
                "AllGather", mybir.AluOpType.bypass,
                replica_groups=REPLICA_GROUPS,
                ins=[k_in[ds(512 * h, 512), :].opt()], outs=[k_gh[h].opt()],
            )


    # ---- V (token-major) + ones column, then all-gather ------------------- #
    _sc_v = nc.named_scope("vphase"); _sc_v.__enter__()
    vw = []
    for kk in range(KK):
        # tag shared with the proj slabs (same shape/dtype, disjoint lifetime)
        wt = wpool.tile([128, 2, C], F8, tag=f"wmid{kk}", name="wv")
        nc.sync.dma_start(out=wt[:], in_=p["vw8"][kk])
        vw.append(wt)
    nc.vector.memset(v_pre[:, :, :, DH : DH + 1], 1.0)
    for tcn in range(T // 128):
        accs = [ps1.tile([128, 512], F32, tag="mm", bufs=4, name=f"vacc{tcn}_{i}") for i in range(2)]
        for kk in range(KK):
            for vf in range(2):
                nc.tensor.matmul(
                    accs[vf][:], h1[:, ds(2 * kk, 2), ds(128 * tcn, 128)],
                    vw[kk][:, :, ds(512 * vf, 512)],
                    start=(kk == 0), stop=False, skip_group_check=True,
                    perf_mode=DR,
                )
        for vf in range(2):
            nc.tensor.matmul(
                accs[vf][:], ones1_128[:], vb2[:, ds(512 * vf, 512)],
                start=False, stop=True, skip_group_check=True,
            )
            nc.vector.tensor_scalar_mul(
                out=v_pre[:, tcn, ds(8 * vf, 8), 0:DH],
                in0=accs[vf][:].rearrange("p (h d) -> p h d", d=DH),
                scalar1=1.0 / WS,
            )
    nc.sync.dma_start(
        out=v_in.rearrange("(tc p) h d -> p tc h d", p=128), in_=v_pre[:]
    )
    _sc_v.__exit__(None, None, None)
    with nc.named_scope("agV"):
        nc.gpsimd.collective_compute(
            "AllGather", mybir.AluOpType.bypass, replica_groups=REPLICA_GROUPS,
            ins=[v_in.opt()], outs=[v_g.opt()],
        )
    # full V into SBUF once, 2080B lines: [p, r, tc, h, d]
    for rr in range(GROUP):
        nc.sync.dma_start(
            out=vfull.rearrange("p r tc h d -> r p tc (h d)")[rr],
            in_=v_g.rearrange("r (tc p) h d -> r p tc (h d)", p=128)[rr],
        )

    # ---- Q tiles last (overlap the gathers) ------------------------------- #
    _sc_q = nc.named_scope("qtiles"); _sc_q.__enter__()
    qk_block(list(range(0, 8)), True)
    rqn = make_rnorm(ssq_q, True)
    rnorm_apply(rqn, rq_d, qe, ps1)
    _sc_q.__exit__(None, None, None)

    ps1.release()

    # ============================ attention ================================ #
    ps2 = tc.alloc_tile_pool(name="ps2", bufs=1, space="PSUM")
    ptpool = tc.alloc_tile_pool(name="ptpool", bufs=3)
    kpool = tc.alloc_tile_pool(name="kpool", bufs=3)

    den = stats.tile([16, T], F32, tag="den")     # softmax denominators
    KT = L // 128  # 16 key tiles
    _sc_at = nc.named_scope("attn"); _sc_at.__enter__()
    for pr in range(8):  # head pairs
        # stream this head-pair's K slice from the gathered DRAM buffer
        kt_sb = kpool.tile([128, GROUP, T], BF16, tag="k")
        hh, tl = divmod(pr, 4)
        nc.sync.dma_start(
            out=kt_sb[:],
            in_=k_gh[hh, :, tl].rearrange("r p q -> p r q"),
        )
        o_ps = [ps2.tile([DH + 1, T], F32, tag="o", bufs=4, name=f"o{pr}_{i}") for i in range(2)]
        for kt in range(KT):
            r, ktc = divmod(kt, 4)
            sp = ps2.tile([128, 2 * T], F32, tag="sc", bufs=2)
            for j in range(2):
                nc.tensor.matmul(
                    sp[:, ds(T * j, T)],
                    kt_sb[ds(64 * j, 64), r, ds(128 * ktc, 128)],
                    qe[ds(64 * j, 64), pr, :],
                    start=True, stop=True,
                )
            pt = ptpool.tile([128, 2 * T], BF16, tag="pt")
            nc.scalar.activation(out=pt[:], in_=sp[:], func=AF.Exp)
            # bias multiply split ~2:1 across DVE and GpSimd (GpSimd measures
            # ~2x slower per element, so it gets the smaller share)
            for j in range(2):
                eng = nc.gpsimd if (2 * kt + j) % 3 == 2 else nc.vector
                eng.tensor_tensor(out=pt[:, ds(T * j, T)],
                                  in0=pt[:, ds(T * j, T)],
                                  in1=eb[:, kt, :], op=mybir.AluOpType.mult)
            for j in range(2):
                nc.tensor.matmul(
                    o_ps[j][:], vfull[:, r, ktc, 2 * pr + j, :],
                    pt[:, ds(T * j, T)],
                    start=(kt == 0), stop=(kt == KT - 1), skip_group_check=True,
                )
        for j in range(2):
            # park the unscaled numerator + stage the denominator row
            # (engine APs need 32-aligned partition bases, so bounce the
            #  denominator rows through DRAM to collect them on 16 partitions)
            nc.vector.tensor_copy(out=anum[ds(64 * j, 64), pr, :],
                                  in_=o_ps[j][0:DH, :])
            dstage = work.tile([1, T], F32, tag="dstage", bufs=2, name="dstage")
            nc.vector.tensor_copy(out=dstage[:], in_=o_ps[j][ds(DH, 1), :])
            nc.sync.dma_start(out=den_d[ds(2 * pr + j, 1), :], in_=dstage[:])
    nc.sync.dma_start(out=den[:], in_=den_d[:])

    # batched softmax normalization: one reciprocal, selector-matmul bcast
    recs = stats.tile([16, T], BF16, tag="recs")
    with nc.allow_low_precision(reason="softmax denom recip in bf16 is fine at 2e-2 tol"):
        nc.vector.reciprocal(out=recs[:], in_=den[:])
    for pr in range(8):
        bcp = ps2.tile([128, T], F32, tag="sc", bufs=2, name="bcp")
        nc.tensor.matmul(bcp[:], hselT[:, pr, :], recs[:], start=True, stop=True)
        nc.vector.tensor_tensor(
            out=attn[:, pr, :], in0=anum[:, pr, :], in1=bcp[:],
            op=mybir.AluOpType.mult,
        )

    _sc_at.__exit__(None, None, None)
    kpool.release()
    ptpool.release()
    ps2.release()

    # ============================ proj + FFN =============================== #
    ps3 = tc.alloc_tile_pool(name="ps3", bufs=1, space="PSUM")

    def dense8(wname, nkk, Mtiles, rhs, consume, wtag, resident, mm_bufs=8,
               mgrp=4):
        """fp8 DoubleRow dense layer: out[m] += w8[kk][:,:,m].T2 @ rhs pairs."""
        if resident:
            slabs = []
            for kk in range(nkk):
                wt = wpool.tile([128, 2, 128 * Mtiles], F8,
                                tag=f"wmid{kk}", name=wtag)
                nc.sync.dma_start(out=wt[:], in_=p[wname][kk])
                slabs.append(wt)
            for mg in range(0, Mtiles, mgrp):
                nsub = min(mgrp, Mtiles - mg)
                accs = [ps3.tile([128, T], F32, tag="mm", bufs=mm_bufs,
                                 name=f"d{wtag}{mg}_{i}") for i in range(nsub)]
                for kk in range(nkk):
                    for i in range(nsub):
                        nc.tensor.matmul(
                            accs[i][:],
                            slabs[kk][:, :, ds(128 * (mg + i), 128)],
                            rhs[:, ds(2 * kk, 2), :],
                            start=(kk == 0), stop=(kk == nkk - 1), perf_mode=DR,
                        )
                for i in range(nsub):
                    consume(mg + i, accs[i])
        else:
            # stream slabs: single m-group covering all Mtiles (kk outer)
            accs = [ps3.tile([128, T], F32, tag="mm", bufs=mm_bufs,
                             name=f"d{wtag}_{i}") for i in range(Mtiles)]
            for kk in range(nkk):
                wt = wpool.tile([128, 2, 128 * Mtiles], F8, tag=wtag, bufs=3,
                                name=wtag)
                nc.sync.dma_start(out=wt[:], in_=p[wname][kk])
                for i in range(Mtiles):
                    nc.tensor.matmul(
                        accs[i][:], wt[:, :, ds(128 * i, 128)],
                        rhs[:, ds(2 * kk, 2), :],
                        start=(kk == 0), stop=(kk == nkk - 1), perf_mode=DR,
                    )
            for i in range(Mtiles):
                consume(i, accs[i])

    # proj -> x2 = xb' + proj_out * g1   (proj_b*g1 was pre-folded into xb)
    def proj_consume(m, acc):
        nc.vector.scalar_tensor_tensor(
            out=x2[:, m, :], in0=acc[:], scalar=modg[:, ds(m, 1)],
            in1=xb[:, m, :],
            op0=mybir.AluOpType.mult, op1=mybir.AluOpType.add,
        )

    with nc.named_scope("proj"):
        dense8("projw8", KK, CT, attn, proj_consume, "wpj", resident=True)

    with nc.named_scope("ln2"):
        layernorm(x2, h2, 24, 40, ps3, 8)  # s2 cols 24..31, sh2 cols 40..47
    # fold fc2_b*g2 into the residual now so fc2_consume is a single op
    for t in range(CT):
        nc.vector.tensor_scalar_add(out=x2[:, t, :], in0=x2[:, t, :],
                                    scalar1=fbg2[:, ds(t, 1)])

    def fc1_consume(m, acc):
        if not sim_gelu:
            nc.scalar.activation(
                out=gact[:, m, :], in_=acc[:], func=AF.Gelu_apprx_tanh,
                bias=f1b[:, ds(m, 1)], scale=1.0 / WS,
            )
            return
        # simulator fallback: explicit tanh-approx gelu
        xs = work.tile([128, T], F32, tag="d1", name="xs")
        nc.scalar.activation(out=xs[:], in_=acc[:], func=AF.Identity,
                             bias=f1b[:, ds(m, 1)], scale=1.0 / WS)
        t1 = work.tile([128, T], F32, tag="gsim", name="t1")
        nc.vector.tensor_tensor(out=t1[:], in0=xs[:], in1=xs[:],
                                op=mybir.AluOpType.mult)
        nc.vector.tensor_tensor(out=t1[:], in0=t1[:], in1=xs[:],
                                op=mybir.AluOpType.mult)
        nc.vector.scalar_tensor_tensor(
            out=t1[:], in0=t1[:], scalar=0.044715, in1=xs[:],
            op0=mybir.AluOpType.mult, op1=mybir.AluOpType.add,
        )
        nc.scalar.activation(out=t1[:], in_=t1[:], func=AF.Tanh,
                             scale=0.7978845608028654)
        nc.vector.tensor_scalar(
            out=t1[:], in0=t1[:], scalar1=0.5, scalar2=0.5,
            op0=mybir.AluOpType.mult, op1=mybir.AluOpType.add,
        )
        nc.vector.tensor_tensor(out=gact[:, m, :], in0=t1[:], in1=xs[:],
                                op=mybir.AluOpType.mult)

    with nc.named_scope("fc1"):
        # two half-layers of 16 m-tiles each; 4 resident slabs per half
        for half in range(2):
            slabs = []
            for kk in range(KK):
                wt = wpool.tile([128, 2, 2048], F8, tag=f"wbig{kk}", name="wf1")
                nc.sync.dma_start(
                    out=wt[:], in_=p["fc1w8"][kk][:, :, ds(2048 * half, 2048)]
                )
                slabs.append(wt)
            for mg in range(0, 16, 4):
                accs = [ps3.tile([128, T], F32, tag="mm", bufs=8,
                                 name=f"df1{half}_{mg}_{i}") for i in range(4)]
                for kk in range(KK):
                    for i in range(4):
                        nc.tensor.matmul(
                            accs[i][:],
                            slabs[kk][:, :, ds(128 * (mg + i), 128)],
                            h2[:, ds(2 * kk, 2), :],
                            start=(kk == 0), stop=(kk == KK - 1), perf_mode=DR,
                        )
                for i in range(4):
                    fc1_consume(16 * half + mg + i, accs[i])

    def fc2_consume(m, acc):
        d1 = work.tile([128, T], F32, tag="d1")
        nc.vector.scalar_tensor_tensor(
            out=d1[:], in0=acc[:], scalar=modg[:, ds(8 + m, 1)], in1=x2[:, m, :],
            op0=mybir.AluOpType.mult, op1=mybir.AluOpType.add,
        )
        nc.sync.dma_start(
            out=out_d.rearrange("(t p) q -> t p q", p=128)[m], in_=d1[:]
        )

    with nc.named_scope("fc2"):
        dense8("fc2w8", DFF // 256, CT, gact, fc2_consume, "wf2",
               resident=False, mm_bufs=8)

    ps3.release()
    for pool in (dram, wpool, stats, work, persist, const):
        pool.release()


# --------------------------------------------------------------------------- #
# host side: shard, run, gather
# --------------------------------------------------------------------------- #

def _shard(inputs):
    bf = ml_dtypes.bfloat16
    f8 = ml_dtypes.float8_e4m3
    x = np.asarray(inputs["x"], np.float32)
    cond = np.asarray(inputs["cond_BD"], np.float32)
    bias = np.asarray(inputs["attn_bias"], np.float32)[0, 0]  # [L, L]
    qkv_w = np.asarray(inputs["qkv_w"], np.float32)
    q_bias = np.asarray(inputs["q_bias"], np.float32)
    v_bias = np.asarray(inputs["v_bias"], np.float32)
    scale_mul = np.asarray(inputs["scale_mul"], np.float32).reshape(H)
    proj_w = np.asarray(inputs["proj_w"], np.float32)
    proj_b = np.asarray(inputs["proj_b"], np.float32)
    fc1_w = np.asarray(inputs["fc1_w"], np.float32)
    fc1_b = np.asarray(inputs["fc1_b"], np.float32)
    fc2_w = np.asarray(inputs["fc2_w"], np.float32)
    fc2_b = np.asarray(inputs["fc2_b"], np.float32)
    ada_w = np.asarray(inputs["ada_w"], np.float32)
    ada_b = np.asarray(inputs["ada_b"], np.float32)

    hsel = np.zeros((128, CT, 16), np.float32)
    for t in range(CT):
        hsel[:64, t, 2 * t] = 1.0
        hsel[64:, t, 2 * t + 1] = 1.0
    hselT = np.ascontiguousarray(hsel.transpose(2, 1, 0))  # [16, CT, 128]
    pairsel_np = np.zeros((2, 128), np.float32)
    pairsel_np[0, :64] = 1.0
    pairsel_np[1, 64:] = 1.0

    def pair_w(wT, M):
        # [C_in, M] -> [C_in//256, 128, 2, M] fp8 DoubleRow slabs, scaled x WS
        nkk = wT.shape[0] // 256
        w = np.clip(wT * WS, -240.0, 240.0).reshape(nkk, 2, 128, M)
        return np.ascontiguousarray(w.transpose(0, 2, 1, 3)).astype(f8)

    qkvT = qkv_w.T  # [C, 3C]
    shared = {
        "adawT": np.ascontiguousarray(
            ada_w.T.reshape(CT, 128, 3, 2048).transpose(2, 0, 1, 3)
        ).astype(bf),
        "qkw8": pair_w(qkvT[:, : 2 * C], 2 * C),
        "vw8": pair_w(qkvT[:, 2 * C :], C),
        "projw8": pair_w(proj_w.T, C),
        "fc1w8": pair_w(fc1_w.T, DFF),
        "fc2w8": pair_w(fc2_w.T, C),

        "adab48": np.ascontiguousarray(ada_b.reshape(48, 128).T),
        "qb8": np.ascontiguousarray(q_bias.reshape(CT, 128).T),
        "vb2": (v_bias.reshape(1, C) * WS).astype(bf),
        "pb8": np.ascontiguousarray(proj_b.reshape(CT, 128).T),
        "f1b": np.ascontiguousarray(fc1_b.reshape(DFF // 128, 128).T),
        "f2b": np.ascontiguousarray(fc2_b.reshape(CT, 128).T),
        "smv": scale_mul.reshape(16, 1).copy(),
        "ones128": np.ones((128, 128), np.float32).astype(bf),
        "hsel": hsel.astype(bf),
        "hselT": hselT.astype(bf),
        "ones1_128": np.ones((1, 128), np.float32).astype(bf),
        "pairsel": pairsel_np.astype(bf),
        "eye48": np.eye(48, dtype=np.float32),
    }

    in_maps = []
    for core in range(NCORES):
        g, r = divmod(core, GROUP)
        qs = slice(T * r, T * (r + 1))
        m = dict(shared)
        m["xb"] = np.ascontiguousarray(
            x[g, qs].T.reshape(CT, 128, T)
        ).astype(bf)
        m["cond8"] = np.ascontiguousarray(cond[g].reshape(8, 128).T)
        m["biasT"] = np.ascontiguousarray(
            bias[qs].T.reshape(16, 128, T)
        ).astype(bf)

        in_maps.append(m)
    return in_maps


def kernel(**inputs):
    if "nc" not in _CACHE:
        _CACHE["nc"] = _build()
    nc = _CACHE["nc"]
    in_maps = _shard(inputs)
    try:
        res = bass_utils.run_bass_kernel_spmd(
            nc, in_maps, core_ids=list(range(NCORES))
        )
    except Exception:
        # transient device-state hiccup (seen after profiled runs); retry once
        res = bass_utils.run_bass_kernel_spmd(
            nc, in_maps, core_ids=list(range(NCORES))
        )
    out = np.empty((B, L, C), np.float32)
    for core in range(NCORES):
        g, r = divmod(core, GROUP)
        out[g, T * r : T * (r + 1)] = res.results[core]["out"].T
    return out


# revision 69
# speedup vs baseline: 1.0865x; 1.0283x over previous
"""AdaLN self-attention block (B=2, L=2048, C=1024, H=16, DFF=4096) on 8 TRN2 cores.

Sharding: DP=2 over batch (cores 0-3 -> batch 0, cores 4-7 -> batch 1),
sequence-parallel 4-way within each group (512 query tokens per core).
Each core holds full weights, computes q/k/v for its own 512 tokens,
all-gathers normalized K and V (with an appended ones column for the softmax
denominator) within its 4-core group, runs full attention for its queries,
then proj + FFN locally on its token slice. Host concatenates the slices.

Dense matmuls (qkv / v / proj / fc1 / fc2) run in fp8e4m3 with DoubleRow
perf mode (256-deep contraction per pass, ~1.7x PE throughput). Weights are
scaled x64 on the host so they sit in e4m3's normal range; the descale by
1/64 is folded into each consumer's existing scalar op. Activations feeding
those matmuls (h1, h2, attn, gelu) are written as fp8 directly by their
producing ops. Attention QK / AV and the residual stream stay bf16.

Everything on-chip is feature-major ([C, tokens]); the host pre-transposes
activations/weights so no on-device transposes are needed (except a tiny
48x128 one for the adaLN modulation vector).
"""

import os
import sys

for _p in ("/opt/trn_rl_repo", os.path.expanduser("~/.axon_site/_ro/trn_rl_repo")):
    if os.path.isdir(_p) and _p not in sys.path:
        sys.path.insert(0, _p)

import numpy as np
import ml_dtypes

import concourse.bass as bass
import concourse.tile as tile
from concourse import mybir
from concourse.bass import ds, ts
from concourse import bass_utils

BF16 = mybir.dt.bfloat16
F32 = mybir.dt.float32
F8 = mybir.dt.float8e4
AF = mybir.ActivationFunctionType
DR = mybir.MatmulPerfMode.DoubleRow

B, L, C, H, DH, DFF, D = 2, 2048, 1024, 16, 64, 4096, 1024
NCORES = 8
GROUP = 4          # cores per batch group
T = L // GROUP     # 512 query tokens per core
CT = C // 128      # 8 feature tiles
KK = C // 256      # 4 DoubleRow contraction slabs over C
ADA_SLICE = 6 * C // GROUP  # 1536 adaLN outputs per core
EPS = 1e-6
MAX_SCALE_MUL = float(np.log(100.0))
WS = 64.0          # fp8 weight scale
REPLICA_GROUPS = [[0, 1, 2, 3], [4, 5, 6, 7]]

_CACHE = {}


# --------------------------------------------------------------------------- #
# graph construction
# --------------------------------------------------------------------------- #

def _build(sim_gelu=False, split_waits=True):
    nc = bass.Bass(
        "TRN2", target_bir_lowering=False, debug=False, num_devices=NCORES
    )

    def inp(name, shape, dt):
        return nc.dram_tensor(name, shape, dt, kind="ExternalInput").ap()

    p = {
        "xb": inp("xb", [CT, 128, T], BF16),      # x^T slice, tiled, bf16
        "cond8": inp("cond8", [128, 8], F32),     # cond feature-major
        "biasT": inp("biasT", [16, 128, T], BF16),  # exp-bias source, tiled
        "qkw8": inp("qkw8", [KK, 128, 2, 2 * C], F8),
        "vw8": inp("vw8", [KK, 128, 2, C], F8),
        "projw8": inp("projw8", [KK, 128, 2, C], F8),
        "fc1w8": inp("fc1w8", [KK, 128, 2, DFF], F8),
        "fc2w8": inp("fc2w8", [DFF // 256, 128, 2, C], F8),
        "adawT": inp("adawT", [3, CT, 128, 2048], BF16),
        "adab48": inp("adab48", [128, 48], F32),
        "qb8": inp("qb8", [128, CT], F32),
        "vb2": inp("vb2", [1, C], BF16),          # host-scaled x WS
        "pb8": inp("pb8", [128, CT], F32),
        "f1b": inp("f1b", [128, DFF // 128], F32),
        "f2b": inp("f2b", [128, CT], F32),
        "smv": inp("smv", [16, 1], F32),
        "ones128": inp("ones128", [128, 128], BF16),
        "hsel": inp("hsel", [128, CT, 16], BF16),
        "hselT": inp("hselT", [16, CT, 128], BF16),
        "ones1_128": inp("ones1_128", [1, 128], BF16),
        "pairsel": inp("pairsel", [2, 128], BF16),
        "eye48": inp("eye48", [48, 48], F32),
    }
    out = nc.dram_tensor("out", [C, T], F32, kind="ExternalOutput").ap()

    with tile.TileContext(nc) as tc:
        _emit(nc, tc, p, out, sim_gelu)
    if split_waits:
        _split_waits(nc)
    return nc


_SPLIT_TYPES = {
    "InstTensorTensor", "InstTensorScalarPtr", "InstReciprocal",
    "InstTensorCopy", "InstActivation", "InstTensorReduce", "InstMemset",
    "InstMatmult", "InstLdweights", "InstCopyPredicated", "InstBnStats",
    "InstBnAggr", "InstStreamTranspose", "InstDMACopy", "InstDrain",
    "InstCollectiveCompute",
}


def _split_waits(nc, max_waits=1):
    """Walrus TPB codegen rejects >1 sync-wait on compute instructions;
    hoist extras onto standalone EventSemaphore waits on the same engine."""
    for bb in nc.main_func.blocks:
        new = []
        changed = False
        for ins in bb.instructions:
            si = getattr(ins, "sync_info", None)
            if (
                si is not None
                and si.on_wait
                and len(si.on_wait) > max_waits
                and type(ins).__name__ in _SPLIT_TYPES
            ):
                waits = list(si.on_wait)
                for i, w in enumerate(waits[:-max_waits]):
                    ws = mybir.InstEventSemaphore(
                        name=f"{ins.name}_w{i}", ins=[], outs=[]
                    )
                    ws.engine = ins.engine
                    ws.sync_info = mybir.SyncInfo(on_wait=[w], on_update=[])
                    new.append(ws)
                ins.sync_info = mybir.SyncInfo(
                    on_wait=waits[-max_waits:], on_update=list(si.on_update)
                )
                changed = True
            new.append(ins)
        if changed:
            bb.instructions = new


def _emit(nc, tc, p, out_d, sim_gelu=False):

    # ---- persistent SBUF pools -------------------------------------------- #
    const = tc.alloc_tile_pool(name="const", bufs=1)
    persist = tc.alloc_tile_pool(name="persist", bufs=1)
    work = tc.alloc_tile_pool(name="work", bufs=4)
    stats = tc.alloc_tile_pool(name="stats", bufs=1)
    wpool = tc.alloc_tile_pool(name="wpool", bufs=1)
    dram = tc.alloc_tile_pool(name="dram", bufs=1, space="DRAM")

    # ---- constants / small inputs to SBUF --------------------------------- #
    def load_const(name, shape, dt):
        t = const.tile(shape, dt, tag=name, name=name)
        nc.sync.dma_start(out=t[:], in_=p[name])
        return t

    # ada path inputs first: its collective is the first serialization point
    cond8 = load_const("cond8", [128, 8], F32)
    adab48 = load_const("adab48", [128, 48], F32)
    eye48 = load_const("eye48", [48, 48], F32)
    ones128 = load_const("ones128", [128, 128], BF16)
    hsel = load_const("hsel", [128, CT, 16], BF16)
    hselT = load_const("hselT", [16, CT, 128], BF16)
    ones1_128 = load_const("ones1_128", [1, 128], BF16)
    pairsel = load_const("pairsel", [2, 128], BF16)
    qb8 = load_const("qb8", [128, CT], F32)
    vb2 = load_const("vb2", [1, C], BF16)
    pb8 = load_const("pb8", [128, CT], F32)
    f1b = load_const("f1b", [128, DFF // 128], F32)
    f2b = load_const("f2b", [128, CT], F32)
    smv_in = load_const("smv", [16, 1], F32)

    # ---- DRAM bounce buffers ---------------------------------------------- #
    ada_ls = [dram.tile([1, 2048], F32, tag=f"ada_l{i}", name=f"ada_l{i}")
              for i in range(3)]
    k_ins = [dram.tile([512, T], BF16, tag=f"k_in{h}", name=f"k_in{h}")
             for h in range(2)]
    k_gs = [dram.tile([GROUP, 4, 128, T], BF16, tag=f"k_g{h}", name=f"k_g{h}")
            for h in range(2)]
    v_in = dram.tile([T, H, DH + 1], BF16, tag="v_in")
    v_g = dram.tile([GROUP, T, H, DH + 1], BF16, tag="v_g")
    rq_d = dram.tile([16, T], BF16, tag="rq_d")
    rk_d = dram.tile([16, T], BF16, tag="rk_d")
    den_d = dram.tile([16, T], F32, tag="den_d")

    # ---- adaLN: silu(cond), then full 6C ada vector computed redundantly -- #
    sig = work.tile([128, 8], F32, tag="w8")
    nc.scalar.activation(out=sig[:], in_=cond8[:], func=AF.Exp, scale=-1.0)
    nc.vector.tensor_scalar_add(out=sig[:], in0=sig[:], scalar1=1.0)
    nc.vector.reciprocal(out=sig[:], in_=sig[:])
    silu = work.tile([128, 8], BF16, tag="w8b")
    nc.vector.tensor_tensor(
        out=silu[:], in0=sig[:], in1=cond8[:], op=mybir.AluOpType.mult
    )

    # Every core computes the full 6C adaLN vector redundantly: a collective
    # here costs ~55us of trigger latency, the redundant matmuls only ~25us.
    # Chunk order (s1/s2 -> sh1/sh2 -> g1/g2) so LN1's modulation is ready
    # first and the g1/g2 gates (needed only at proj/fc2) stream last.
    mod = persist.tile([128, 48], F32, tag="mod")
    modg = stats.tile([128, 16], F32, tag="modg")
    pbg1 = stats.tile([128, CT], F32, tag="pbg1")
    fbg2 = stats.tile([128, CT], F32, tag="fbg2")
    ps0 = tc.alloc_tile_pool(name="ps0", bufs=1, space="PSUM")
    _sc_ada = nc.named_scope("ada"); _sc_ada.__enter__()
    for ngp in (1, 2, 0):
        aps = [ps0.tile([1, 1024], F32, tag="ada", bufs=2, name=f"aps{ngp}_{i}")
               for i in range(2)]
        for k in range(CT):
            wt = wpool.tile([128, 2, 1024], BF16, tag="wada", bufs=2,
                            name="wada")
            nc.sync.dma_start(
                out=wt.rearrange("p i c -> p (i c)"),
                in_=p["adawT"][ngp, k],
            )
            for i in range(2):
                for s in range(2):
                    nc.tensor.matmul(
                        aps[i][0:1, ds(512 * s, 512)], silu[:, ds(k, 1)],
                        wt[:, i, ds(512 * s, 512)],
                        start=(k == 0), stop=(k == CT - 1),
                        skip_group_check=True,
                    )
        for i in range(2):
            aw = work.tile([1, 1024], F32, tag="w1x512", bufs=2, name="aw")
            nc.vector.tensor_copy(out=aw[:], in_=aps[i][:])
            nc.sync.dma_start(
                out=ada_ls[ngp][0, ds(1024 * i, 1024)], in_=aw[:]
            )
        # build this ngp's 16 mod columns: [16,128] load -> PE transpose
        atm = work.tile([16, 128], F32, tag="ada_tm", name="atm")
        nc.sync.dma_start(
            out=atm[:],
            in_=ada_ls[ngp].rearrange("g n -> (g n)").rearrange(
                "(j p) -> j p", p=128
            ),
        )
        modps = ps0.tile([128, 16], F32, tag="adat", bufs=2, name="modps")
        nc.tensor.transpose(modps[:], atm[:], eye48[0:16, 0:16])
        nc.vector.tensor_tensor(
            out=mod[:, ds(16 * ngp, 16)], in0=modps[:],
            in1=adab48[:, ds(16 * ngp, 16)], op=mybir.AluOpType.add,
        )
        if ngp == 1:  # s1, s2 chunks get +1
            nc.vector.tensor_scalar_add(out=mod[:, 16:32], in0=mod[:, 16:32],
                                        scalar1=1.0)
        if ngp == 0:  # descaled g1/g2 copies for fp8 PSUM consumers
            nc.vector.tensor_scalar_mul(out=modg[:], in0=mod[:, 0:16],
                                        scalar1=1.0 / WS)
            nc.vector.tensor_tensor(out=pbg1[:], in0=pb8[:], in1=mod[:, 0:8],
                                    op=mybir.AluOpType.mult)
            nc.vector.tensor_tensor(out=fbg2[:], in0=f2b[:], in1=mod[:, 8:16],
                                    op=mybir.AluOpType.mult)
    ps0.release()
    _sc_ada.__exit__(None, None, None)

    # ============================ phase 1 PSUM ============================= #
    ps1 = tc.alloc_tile_pool(name="ps1", bufs=1, space="PSUM")

    # ---- persistent activations ------------------------------------------- #
    xb = persist.tile([128, CT, T], BF16, tag="big_d")       # x^T bf16
    nc.sync.dma_start(out=xb[:], in_=p["xb"].rearrange("t p q -> p t q"))

    h1 = persist.tile([128, CT, T], F8, tag="big_a")         # LN1-mod, fp8
    qe = persist.tile([128, CT, T], BF16, tag="big_b")       # q (later normed)
    ke = persist.tile([128, CT, T], BF16, tag="big_c")       # k (later normed)
    v_pre = persist.tile([128, T // 128, H, DH + 1], BF16, tag="vpre")
    eb = persist.tile([128, L // 128, T], BF16, tag="eb")    # exp(bias^T)
    attn = persist.tile([128, CT, T], F8, tag="big_a")       # fp8 probs@V
    anum = persist.tile([128, CT, T], BF16, tag="vpre")      # unscaled attn out
    x2 = persist.tile([128, CT, T], BF16, tag="big_c")
    h2 = persist.tile([128, CT, T], F8, tag="big_b")
    gact = persist.tile([128, DFF // 128, T], F8, tag="big_d")
    vfull = persist.tile([128, GROUP, T // 128, H, DH + 1], BF16, tag="vfull")

    # ---- scale_mul -> smv = exp(min(scale_mul, log 100)) ------------------ #
    eps128 = const.tile([128, 1], F32, tag="eps128")
    nc.vector.memset(eps128[:], EPS)
    smv = stats.tile([16, 1], F32, tag="smv")
    nc.vector.tensor_scalar_min(out=smv[:], in0=smv_in[:], scalar1=MAX_SCALE_MUL)
    nc.scalar.activation(out=smv[:], in_=smv[:], func=AF.Exp)

    # ---- layernorm helper (feature-major, partition sums via ones matmul) - #
    def layernorm(src, dst, s_col, sh_col, psp, mm_bufs):
        s1 = psp.tile([128, T], F32, tag="mm", bufs=mm_bufs)
        s2 = psp.tile([128, T], F32, tag="mm", bufs=mm_bufs)
        for t in range(CT):
            sq = work.tile([128, T], BF16, tag="sq")
            nc.vector.tensor_tensor(
                out=sq[:], in0=src[:, t, :], in1=src[:, t, :], op=mybir.AluOpType.mult
            )
            nc.tensor.matmul(s1[:], ones128[:], src[:, t, :],
                             start=(t == 0), stop=(t == CT - 1), skip_group_check=True)
            nc.tensor.matmul(s2[:], ones128[:], sq[:],
                             start=(t == 0), stop=(t == CT - 1), skip_group_check=True)
        meanb = stats.tile([128, T], F32, tag="meanb")
        nc.vector.tensor_scalar_mul(out=meanb[:], in0=s1[:], scalar1=1.0 / C)
        m2 = stats.tile([128, T], F32, tag="m2")
        nc.vector.tensor_tensor(out=m2[:], in0=meanb[:], in1=meanb[:],
                                op=mybir.AluOpType.mult)
        varb = stats.tile([128, T], F32, tag="varb")
        nc.vector.scalar_tensor_tensor(
            out=varb[:], in0=s2[:], scalar=1.0 / C, in1=m2[:],
            op0=mybir.AluOpType.mult, op1=mybir.AluOpType.subtract,
        )
        # rstd = exp(-0.5 * ln(var + eps))   (stays in the exp/ln table set)
        nc.scalar.activation(out=varb[:], in_=varb[:], func=AF.Ln, bias=eps128[:])
        rstdb = stats.tile([128, T], F32, tag="rstdb")
        nc.scalar.activation(out=rstdb[:], in_=varb[:], func=AF.Exp, scale=-0.5)
        for t in range(CT):
            d1 = work.tile([128, T], F32, tag="d1")
            nc.vector.tensor_tensor(out=d1[:], in0=src[:, t, :], in1=meanb[:],
                                    op=mybir.AluOpType.subtract)
            nc.vector.tensor_tensor(out=d1[:], in0=d1[:], in1=rstdb[:],
                                    op=mybir.AluOpType.mult)
            nc.vector.tensor_scalar(
                out=dst[:, t, :], in0=d1[:],
                scalar1=mod[:, ds(s_col + t, 1)], scalar2=mod[:, ds(sh_col + t, 1)],
                op0=mybir.AluOpType.mult, op1=mybir.AluOpType.add,
            )

    with nc.named_scope("ln1"):
        layernorm(xb, h1, 16, 32, ps1, 4)  # s1 cols 16..23, sh1 cols 32..39
    # fold proj_b*g1 into the residual now so proj_consume is a single op
    for t in range(CT):
        nc.vector.tensor_scalar_add(out=xb[:, t, :], in0=xb[:, t, :],
                                    scalar1=pbg1[:, ds(t, 1)])

    # ---- qkv weights: 4 resident fp8 slabs -------------------------------- #
    qkw = []
    for kk in range(KK):
        # tag shared with the fc1 slabs (same shape/dtype, disjoint lifetime)
        wt = wpool.tile([128, 2, 2 * C], F8, tag=f"wbig{kk}", name="wqk")
        nc.sync.dma_start(out=wt[:], in_=p["qkw8"][kk])
        qkw.append(wt)

    # ---- qkv: K first (so its all-gather overlaps V and Q compute) ------- #
    _sc_qkv = nc.named_scope("qkv"); _sc_qkv.__enter__()
    ssq_q = ps1.tile([16, T], F32, tag="ss", bufs=2)
    ssq_k = ps1.tile([16, T], F32, tag="ss", bufs=2)

    def qk_block(ms, is_q):
        # ms: global m-tile indices into the 2C q/k output (0..7 q, 8..15 k)
        for mg in range(0, len(ms), 4):
            sub = ms[mg:mg + 4]
            accs = [ps1.tile([128, T], F32, tag="mm", bufs=4, name=f"qk{m}")
                    for m in sub]
            for kk in range(KK):
                for i, m in enumerate(sub):
                    nc.tensor.matmul(
                        accs[i][:], qkw[kk][:, :, ds(128 * m, 128)],
                        h1[:, ds(2 * kk, 2), :],
                        start=(kk == 0), stop=(kk == KK - 1), perf_mode=DR,
                    )
            for i, m in enumerate(sub):
                acc = accs[i]
                if is_q:
                    dst = qe[:, m, :]
                    nc.vector.tensor_scalar(
                        out=dst, in0=acc[:], scalar1=1.0 / WS,
                        scalar2=qb8[:, ds(m, 1)],
                        op0=mybir.AluOpType.mult, op1=mybir.AluOpType.add,
                    )
                else:
                    dst = ke[:, m - 8, :]
                    nc.vector.tensor_scalar_mul(out=dst, in0=acc[:],
                                                scalar1=1.0 / WS)
                sq = work.tile([128, T], BF16, tag="sq")
                nc.vector.tensor_tensor(out=sq[:], in0=dst, in1=dst,
                                        op=mybir.AluOpType.mult)
                tgt = ssq_q if is_q else ssq_k
                tm = m % 8
                nc.tensor.matmul(tgt[:], hsel[:, tm, :], sq[:],
                                 start=(tm == 0), stop=(tm == 7),
                                 skip_group_check=True)

    def make_rnorm(ssq, with_sm):
        r = stats.tile([16, T], F32, tag="rn_f")
        nc.vector.tensor_scalar_max(out=r[:], in0=ssq[:], scalar1=1e-24)
        nc.scalar.activation(out=r[:], in_=r[:], func=AF.Ln)
        rb = stats.tile([16, T], BF16, tag="rn_bq" if with_sm else "rn_bk", name="rb")
        nc.scalar.activation(out=rb[:], in_=r[:], func=AF.Exp, scale=-0.5)
        if with_sm:
            nc.vector.tensor_scalar_mul(out=rb[:], in0=rb[:], scalar1=smv[:])
        return rb

    def rnorm_apply(rb, rd_bounce, dst, psp):
        # partition remap [16,T] -> [2,8,T] via a DRAM roundtrip, then a
        # K=2 pairsel matmul broadcasts each head row over its 64 partitions
        nc.sync.dma_start(out=rd_bounce[:], in_=rb[:])
        rn2 = work.tile([2, 8, T], BF16, tag="rn2", bufs=1, name="rn2")
        nc.sync.dma_start(out=rn2[:],
                          in_=rd_bounce.rearrange("(t j) q -> j t q", j=2))
        for t in range(CT):
            bc = psp.tile([128, T], F32, tag="sm", bufs=2, name="bcn")
            nc.tensor.matmul(bc[:], pairsel[:], rn2[:, t, :], start=True, stop=True)
            nc.vector.tensor_tensor(out=dst[:, t, :], in0=dst[:, t, :], in1=bc[:],
                                    op=mybir.AluOpType.mult)

    qk_block(list(range(8, 16)), False)  # K tiles
    rkn = make_rnorm(ssq_k, False)
    rnorm_apply(rkn, rk_d, ke, ps1)
    _sc_qkv.__exit__(None, None, None)

    # ---- K all-gather, split in halves so attention can start early ------- #
    with nc.named_scope("agK"):
        for h in range(2):
            kin_t = k_ins[h].rearrange("(t p) q -> t p q", p=128)
            for tl in range(4):
                nc.sync.dma_start(out=kin_t[tl], in_=ke[:, 4 * h + tl, :])
            nc.gpsimd.collective_compute(
                "AllGather", mybir.AluOpType.bypass,
                replica_groups=REPLICA_GROUPS,
                ins=[k_ins[h].opt()], outs=[k_gs[h].opt()],
            )

    # ---- expbias (needed from attention on; loaded after the K gather) ---- #
    nc.sync.dma_start(out=eb[:], in_=p["biasT"].rearrange("t p q -> p t q"))
    for i in range(4):
        nc.scalar.activation(
            out=eb[:, ds(4 * i, 4), :], in_=eb[:, ds(4 * i, 4), :], func=AF.Exp
        )


    # ---- V (token-major) + ones column, then all-gather ------------------- #
    _sc_v = nc.named_scope("vphase"); _sc_v.__enter__()
    vw = []
    for kk in range(KK):
        # tag shared with the proj slabs (same shape/dtype, disjoint lifetime)
        wt = wpool.tile([128, 2, C], F8, tag=f"wmid{kk}", name="wv")
        nc.sync.dma_start(out=wt[:], in_=p["vw8"][kk])
        vw.append(wt)
    nc.vector.memset(v_pre[:, :, :, DH : DH + 1], 1.0)
    for tcn in range(T // 128):
        accs = [ps1.tile([128, 512], F32, tag="mm", bufs=4, name=f"vacc{tcn}_{i}") for i in range(2)]
        for kk in range(KK):
            for vf in range(2):
                nc.tensor.matmul(
                    accs[vf][:], h1[:, ds(2 * kk, 2), ds(128 * tcn, 128)],
                    vw[kk][:, :, ds(512 * vf, 512)],
                    start=(kk == 0), stop=False, skip_group_check=True,
                    perf_mode=DR,
                )
        for vf in range(2):
            nc.tensor.matmul(
                accs[vf][:], ones1_128[:], vb2[:, ds(512 * vf, 512)],
                start=False, stop=True, skip_group_check=True,
            )
            nc.vector.tensor_scalar_mul(
                out=v_pre[:, tcn, ds(8 * vf, 8), 0:DH],
                in0=accs[vf][:].rearrange("p (h d) -> p h d", d=DH),
                scalar1=1.0 / WS,
            )
    nc.sync.dma_start(
        out=v_in.rearrange("(tc p) h d -> p tc h d", p=128), in_=v_pre[:]
    )
    _sc_v.__exit__(None, None, None)
    with nc.named_scope("agV"):
        nc.gpsimd.collective_compute(
            "AllGather", mybir.AluOpType.bypass, replica_groups=REPLICA_GROUPS,
            ins=[v_in.opt()], outs=[v_g.opt()],
        )
    # full V into SBUF once, 2080B lines: [p, r, tc, h, d]
    for rr in range(GROUP):
        nc.sync.dma_start(
            out=vfull.rearrange("p r tc h d -> r p tc (h d)")[rr],
            in_=v_g.rearrange("r (tc p) h d -> r p tc (h d)", p=128)[rr],
        )

    # ---- Q tiles last (overlap the gathers) ------------------------------- #
    _sc_q = nc.named_scope("qtiles"); _sc_q.__enter__()
    qk_block(list(range(0, 8)), True)
    rqn = make_rnorm(ssq_q, True)
    rnorm_apply(rqn, rq_d, qe, ps1)
    _sc_q.__exit__(None, None, None)

    ps1.release()

    # ============================ attention ================================ #
    ps2 = tc.alloc_tile_pool(name="ps2", bufs=1, space="PSUM")
    ptpool = tc.alloc_tile_pool(name="ptpool", bufs=3)
    kpool = tc.alloc_tile_pool(name="kpool", bufs=3)

    den = stats.tile([16, T], F32, tag="den")     # softmax denominators
    KT = L // 128  # 16 key tiles
    _sc_at = nc.named_scope("attn"); _sc_at.__enter__()
    for pr in range(8):  # head pairs
        # stream this head-pair's K slice from the gathered DRAM buffer
        kt_sb = kpool.tile([128, GROUP, T], BF16, tag="k")
        hh, tl = divmod(pr, 4)
        nc.sync.dma_start(
            out=kt_sb[:],
            in_=k_gs[hh][:, tl].rearrange("r p q -> p r q"),
        )
        o_ps = [ps2.tile([DH + 1, T], F32, tag="o", bufs=2, name=f"o{pr}_{i}") for i in range(2)]
        for kt in range(KT):
            r, ktc = divmod(kt, 4)
            sp = ps2.tile([128, 2 * T], F32, tag="sc", bufs=3)
            for j in range(2):
                nc.tensor.matmul(
                    sp[:, ds(T * j, T)],
                    kt_sb[ds(64 * j, 64), r, ds(128 * ktc, 128)],
                    qe[ds(64 * j, 64), pr, :],
                    start=True, stop=True,
                )
            pt = ptpool.tile([128, 2 * T], BF16, tag="pt", bufs=4)
            nc.scalar.activation(out=pt[:], in_=sp[:], func=AF.Exp)
            # bias multiply split ~2:1 across DVE and GpSimd (GpSimd measures
            # ~2x slower per element, so it gets the smaller share)
            for j in range(2):
                eng = nc.gpsimd if (2 * kt + j) % 3 == 2 else nc.vector
                eng.tensor_tensor(out=pt[:, ds(T * j, T)],
                                  in0=pt[:, ds(T * j, T)],
                                  in1=eb[:, kt, :], op=mybir.AluOpType.mult)
            for j in range(2):
                nc.tensor.matmul(
                    o_ps[j][:], vfull[:, r, ktc, 2 * pr + j, :],
                    pt[:, ds(T * j, T)],
                    start=(kt == 0), stop=(kt == KT - 1), skip_group_check=True,
                )
        for j in range(2):
            # park the unscaled numerator + stage the denominator row
            # (engine APs need 32-aligned partition bases, so bounce the
            #  denominator rows through DRAM to collect them on 16 partitions)
            nc.vector.tensor_copy(out=anum[ds(64 * j, 64), pr, :],
                                  in_=o_ps[j][0:DH, :])
            dstage = work.tile([1, T], F32, tag="dstage", bufs=2, name="dstage")
            nc.vector.tensor_copy(out=dstage[:], in_=o_ps[j][ds(DH, 1), :])
            nc.sync.dma_start(out=den_d[ds(2 * pr + j, 1), :], in_=dstage[:])
    nc.sync.dma_start(out=den[:], in_=den_d[:])

    # batched softmax normalization: one reciprocal, selector-matmul bcast
    recs = stats.tile([16, T], BF16, tag="recs")
    with nc.allow_low_precision(reason="softmax denom recip in bf16 is fine at 2e-2 tol"):
        nc.vector.reciprocal(out=recs[:], in_=den[:])
    for pr in range(8):
        bcp = ps2.tile([128, T], F32, tag="sc", bufs=3, name="bcp")
        nc.tensor.matmul(bcp[:], hselT[:, pr, :], recs[:], start=True, stop=True)
        nc.vector.tensor_tensor(
            out=attn[:, pr, :], in0=anum[:, pr, :], in1=bcp[:],
            op=mybir.AluOpType.mult,
        )

    _sc_at.__exit__(None, None, None)
    kpool.release()
    ptpool.release()
    ps2.release()

    # ============================ proj + FFN =============================== #
    ps3 = tc.alloc_tile_pool(name="ps3", bufs=1, space="PSUM")

    def dense8(wname, nkk, Mtiles, rhs, consume, wtag, resident, mm_bufs=8,
               mgrp=4):
        """fp8 DoubleRow dense layer: out[m] += w8[kk][:,:,m].T2 @ rhs pairs."""
        if resident:
            slabs = []
            for kk in range(nkk):
                wt = wpool.tile([128, 2, 128 * Mtiles], F8,
                                tag=f"wmid{kk}", name=wtag)
                nc.sync.dma_start(out=wt[:], in_=p[wname][kk])
                slabs.append(wt)
            for mg in range(0, Mtiles, mgrp):
                nsub = min(mgrp, Mtiles - mg)
                accs = [ps3.tile([128, T], F32, tag="mm", bufs=mm_bufs,
                                 name=f"d{wtag}{mg}_{i}") for i in range(nsub)]
                for kk in range(nkk):
                    for i in range(nsub):
                        nc.tensor.matmul(
                            accs[i][:],
                            slabs[kk][:, :, ds(128 * (mg + i), 128)],
                            rhs[:, ds(2 * kk, 2), :],
                            start=(kk == 0), stop=(kk == nkk - 1), perf_mode=DR,
                        )
                for i in range(nsub):
                    consume(mg + i, accs[i])
        else:
            # stream slabs: single m-group covering all Mtiles (kk outer)
            accs = [ps3.tile([128, T], F32, tag="mm", bufs=mm_bufs,
                             name=f"d{wtag}_{i}") for i in range(Mtiles)]
            for kk in range(nkk):
                wt = wpool.tile([128, 2, 128 * Mtiles], F8, tag=wtag, bufs=3,
                                name=wtag)
                nc.sync.dma_start(out=wt[:], in_=p[wname][kk])
                for i in range(Mtiles):
                    nc.tensor.matmul(
                        accs[i][:], wt[:, :, ds(128 * i, 128)],
                        rhs[:, ds(2 * kk, 2), :],
                        start=(kk == 0), stop=(kk == nkk - 1), perf_mode=DR,
                    )
            for i in range(Mtiles):
                consume(i, accs[i])

    # proj -> x2 = xb' + proj_out * g1   (proj_b*g1 was pre-folded into xb)
    def proj_consume(m, acc):
        nc.vector.scalar_tensor_tensor(
            out=x2[:, m, :], in0=acc[:], scalar=modg[:, ds(m, 1)],
            in1=xb[:, m, :],
            op0=mybir.AluOpType.mult, op1=mybir.AluOpType.add,
        )

    with nc.named_scope("proj"):
        dense8("projw8", KK, CT, attn, proj_consume, "wpj", resident=True)

    with nc.named_scope("ln2"):
        layernorm(x2, h2, 24, 40, ps3, 8)  # s2 cols 24..31, sh2 cols 40..47
    # fold fc2_b*g2 into the residual now so fc2_consume is a single op
    for t in range(CT):
        nc.vector.tensor_scalar_add(out=x2[:, t, :], in0=x2[:, t, :],
                                    scalar1=fbg2[:, ds(t, 1)])

    def fc1_consume(m, acc):
        if not sim_gelu:
            nc.scalar.activation(
                out=gact[:, m, :], in_=acc[:], func=AF.Gelu_apprx_tanh,
                bias=f1b[:, ds(m, 1)], scale=1.0 / WS,
            )
            return
        # simulator fallback: explicit tanh-approx gelu
        xs = work.tile([128, T], F32, tag="d1", name="xs")
        nc.scalar.activation(out=xs[:], in_=acc[:], func=AF.Identity,
                             bias=f1b[:, ds(m, 1)], scale=1.0 / WS)
        t1 = work.tile([128, T], F32, tag="gsim", name="t1")
        nc.vector.tensor_tensor(out=t1[:], in0=xs[:], in1=xs[:],
                                op=mybir.AluOpType.mult)
        nc.vector.tensor_tensor(out=t1[:], in0=t1[:], in1=xs[:],
                                op=mybir.AluOpType.mult)
        nc.vector.scalar_tensor_tensor(
            out=t1[:], in0=t1[:], scalar=0.044715, in1=xs[:],
            op0=mybir.AluOpType.mult, op1=mybir.AluOpType.add,
        )
        nc.scalar.activation(out=t1[:], in_=t1[:], func=AF.Tanh,
                             scale=0.7978845608028654)
        nc.vector.tensor_scalar(
            out=t1[:], in0=t1[:], scalar1=0.5, scalar2=0.5,
            op0=mybir.AluOpType.mult, op1=mybir.AluOpType.add,
        )
        nc.vector.tensor_tensor(out=gact[:, m, :], in0=t1[:], in1=xs[:],
                                op=mybir.AluOpType.mult)

    with nc.named_scope("fc1"):
        # two half-layers of 16 m-tiles each; 4 resident slabs per half
        for half in range(2):
            slabs = []
            for kk in range(KK):
                wt = wpool.tile([128, 2, 2048], F8, tag=f"wbig{kk}", name="wf1")
                nc.sync.dma_start(
                    out=wt[:], in_=p["fc1w8"][kk][:, :, ds(2048 * half, 2048)]
                )
                slabs.append(wt)
            for mg in range(0, 16, 4):
                accs = [ps3.tile([128, T], F32, tag="mm", bufs=8,
                                 name=f"df1{half}_{mg}_{i}") for i in range(4)]
                for kk in range(KK):
                    for i in range(4):
                        nc.tensor.matmul(
                            accs[i][:],
                            slabs[kk][:, :, ds(128 * (mg + i), 128)],
                            h2[:, ds(2 * kk, 2), :],
                            start=(kk == 0), stop=(kk == KK - 1), perf_mode=DR,
                        )
                for i in range(4):
                    fc1_consume(16 * half + mg + i, accs[i])

    def fc2_consume(m, acc):
        d1 = work.tile([128, T], F32, tag="d1")
        nc.vector.scalar_tensor_tensor(
            out=d1[:], in0=acc[:], scalar=modg[:, ds(8 + m, 1)], in1=x2[:, m, :],
            op0=mybir.AluOpType.mult, op1=mybir.AluOpType.add,
        )
        nc.sync.dma_start(
            out=out_d.rearrange("(t p) q -> t p q", p=128)[m], in_=d1[:]
        )

    with nc.named_scope("fc2"):
        dense8("fc2w8", DFF // 256, CT, gact, fc2_consume, "wf2",
               resident=False, mm_bufs=8)

    ps3.release()
    for pool in (dram, wpool, stats, work, persist, const):
        pool.release()


# --------------------------------------------------------------------------- #
# host side: shard, run, gather
# --------------------------------------------------------------------------- #

def _shard(inputs):
    bf = ml_dtypes.bfloat16
    f8 = ml_dtypes.float8_e4m3
    x = np.asarray(inputs["x"], np.float32)
    cond = np.asarray(inputs["cond_BD"], np.float32)
    bias = np.asarray(inputs["attn_bias"], np.float32)[0, 0]  # [L, L]
    qkv_w = np.asarray(inputs["qkv_w"], np.float32)
    q_bias = np.asarray(inputs["q_bias"], np.float32)
    v_bias = np.asarray(inputs["v_bias"], np.float32)
    scale_mul = np.asarray(inputs["scale_mul"], np.float32).reshape(H)
    proj_w = np.asarray(inputs["proj_w"], np.float32)
    proj_b = np.asarray(inputs["proj_b"], np.float32)
    fc1_w = np.asarray(inputs["fc1_w"], np.float32)
    fc1_b = np.asarray(inputs["fc1_b"], np.float32)
    fc2_w = np.asarray(inputs["fc2_w"], np.float32)
    fc2_b = np.asarray(inputs["fc2_b"], np.float32)
    ada_w = np.asarray(inputs["ada_w"], np.float32)
    ada_b = np.asarray(inputs["ada_b"], np.float32)

    hsel = np.zeros((128, CT, 16), np.float32)
    for t in range(CT):
        hsel[:64, t, 2 * t] = 1.0
        hsel[64:, t, 2 * t + 1] = 1.0
    hselT = np.ascontiguousarray(hsel.transpose(2, 1, 0))  # [16, CT, 128]
    pairsel_np = np.zeros((2, 128), np.float32)
    pairsel_np[0, :64] = 1.0
    pairsel_np[1, 64:] = 1.0

    def pair_w(wT, M):
        # [C_in, M] -> [C_in//256, 128, 2, M] fp8 DoubleRow slabs, scaled x WS
        nkk = wT.shape[0] // 256
        w = np.clip(wT * WS, -240.0, 240.0).reshape(nkk, 2, 128, M)
        return np.ascontiguousarray(w.transpose(0, 2, 1, 3)).astype(f8)

    qkvT = qkv_w.T  # [C, 3C]
    shared = {
        "adawT": np.ascontiguousarray(
            ada_w.T.reshape(CT, 128, 3, 2048).transpose(2, 0, 1, 3)
        ).astype(bf),
        "qkw8": pair_w(qkvT[:, : 2 * C], 2 * C),
        "vw8": pair_w(qkvT[:, 2 * C :], C),
        "projw8": pair_w(proj_w.T, C),
        "fc1w8": pair_w(fc1_w.T, DFF),
        "fc2w8": pair_w(fc2_w.T, C),

        "adab48": np.ascontiguousarray(ada_b.reshape(48, 128).T),
        "qb8": np.ascontiguousarray(q_bias.reshape(CT, 128).T),
        "vb2": (v_bias.reshape(1, C) * WS).astype(bf),
        "pb8": np.ascontiguousarray(proj_b.reshape(CT, 128).T),
        "f1b": np.ascontiguousarray(fc1_b.reshape(DFF // 128, 128).T),
        "f2b": np.ascontiguousarray(fc2_b.reshape(CT, 128).T),
        "smv": scale_mul.reshape(16, 1).copy(),
        "ones128": np.ones((128, 128), np.float32).astype(bf),
        "hsel": hsel.astype(bf),
        "hselT": hselT.astype(bf),
        "ones1_128": np.ones((1, 128), np.float32).astype(bf),
        "pairsel": pairsel_np.astype(bf),
        "eye48": np.eye(48, dtype=np.float32),
    }

    in_maps = []
    for core in range(NCORES):
        g, r = divmod(core, GROUP)
        qs = slice(T * r, T * (r + 1))
        m = dict(shared)
        m["xb"] = np.ascontiguousarray(
            x[g, qs].T.reshape(CT, 128, T)
        ).astype(bf)
        m["cond8"] = np.ascontiguousarray(cond[g].reshape(8, 128).T)
        m["biasT"] = np.ascontiguousarray(
            bias[qs].T.reshape(16, 128, T)
        ).astype(bf)

        in_maps.append(m)
    return in_maps


def kernel(**inputs):
    if "nc" not in _CACHE:
        _CACHE["nc"] = _build()
    nc = _CACHE["nc"]
    in_maps = _shard(inputs)
    try:
        res = bass_utils.run_bass_kernel_spmd(
            nc, in_maps, core_ids=list(range(NCORES))
        )
    except Exception:
        # transient device-state hiccup (seen after profiled runs); retry once
        res = bass_utils.run_bass_kernel_spmd(
            nc, in_maps, core_ids=list(range(NCORES))
        )
    out = np.empty((B, L, C), np.float32)
    for core in range(NCORES):
        g, r = divmod(core, GROUP)
        out[g, T * r : T * (r + 1)] = res.results[core]["out"].T
    return out
